# revision 1
# baseline (speedup 1.0000x reference)
"""AttentionBlock3D (GroupNorm + single-head self-attention + residual) on 8 TRN2 cores.

Sharding: core = (batch b in {0,1}) x (1024-row slice of the 4096 attention rows).
Each core redundantly computes its batch's GroupNorm stats and full K/V
(cheap), and attention + output projection for its own 1024 query rows.
No collectives. The host ROTATES each core's x copy so that its query rows
are always columns 0..1024 (attention is permutation-invariant over keys).

Math folding (all computed on-device from the real inputs; nothing assumes
zero biases):
  hn = x*A + B per channel, with A = gamma*rsqrt(var_g+eps), B = beta - mean_g*A
  q  = (Wq . A) x_q + cq           cq = Wq B + bq   (bias folded into q copy)
  k  = (Wk . A) x                  (k bias cancels in softmax over keys)
  v0 = (Wv . A) x                  cv = Wv B + bv   (rows of softmax sum to 1
                                   => P @ (cv 1^T) = cv 1^T, folded into bpe)
  S^T[m,n] = sum_o k[o,m] q[o,n];  E = exp(S/16);  r[n] = sum_m E[m,n]
  out = (x_q + bpe) + ((Wp^T)^T @ (E^T V)) * (1/r),  bpe = bp + Wp cv
"""

import os
import numpy as np
from contextlib import ExitStack

C = 256          # channels
N = 4096         # spatial positions (16*16*16)
NQ = 1024        # query rows per core
GROUPS = 8
GSIZE = C // GROUPS
EPS = 1e-5
NCH = NQ // 512  # n-chunks per core

_CACHE = {}
LAST_RESULTS = None  # test harness can inspect trace results


def _build_nc(use_f32r=True):
    import concourse.bacc as bacc
    import concourse.bass as bass
    import concourse.tile as tile
    from concourse import mybir

    f32 = mybir.dt.float32
    f32r = mybir.dt.float32r
    AF = mybir.ActivationFunctionType

    fr = f32r if use_f32r else f32
    xbf16 = os.environ.get("BASSK_XBF16", "1") == "1"
    bx = mybir.dt.bfloat16 if xbf16 else fr

    def R(ap):
        return ap

    nc = bacc.Bacc("TRN2", target_bir_lowering=False, debug=False,
                   enable_asserts=False)

    # ---- DRAM I/O (per-core) ----
    xb_d = nc.dram_tensor("xb", [C, N],
                          mybir.dt.bfloat16 if os.environ.get("BASSK_XBF16", "1") == "1" else f32,
                          kind="ExternalInput").ap()
    xq_d = nc.dram_tensor("xq", [C, NQ], f32, kind="ExternalInput").ap()
    wall_d = nc.dram_tensor("wall", [C, 4 * C], f32, kind="ExternalInput").ap()
    small_d = nc.dram_tensor("small", [C, 5 + GROUPS], f32, kind="ExternalInput").ap()
    gmask8_d = nc.dram_tensor("gmask8", [GROUPS, C], f32, kind="ExternalInput").ap()
    out_d = nc.dram_tensor("out", [C, NQ], f32, kind="ExternalOutput").ap()

    with tile.TileContext(nc) as tc, ExitStack() as ctx:
        big = ctx.enter_context(tc.tile_pool(name="big", bufs=1))
        consts = ctx.enter_context(tc.tile_pool(name="consts", bufs=1))
        work = ctx.enter_context(tc.tile_pool(name="work", bufs=3))
        pw = ctx.enter_context(tc.tile_pool(name="pw", bufs=3, space="PSUM"))
        pacc = ctx.enter_context(tc.tile_pool(name="pacc", bufs=3, space="PSUM"))
        pr = ctx.enter_context(tc.tile_pool(name="pr", bufs=1, space="PSUM"))
        pstat = ctx.enter_context(tc.tile_pool(name="pstat", bufs=1, space="PSUM"))

        # ---- constants / small loads (before the big x load) ----
        ones_f32 = consts.tile([128, 128], f32)
        nc.vector.memset(ones_f32, 1.0)
        ones128 = consts.tile([128, 128], fr)
        nc.vector.tensor_copy(ones128, ones_f32)
        # eps8 = Sqrt(EPS^2) on ACT: forces the Sqrt act-table load to run at
        # t~0 (gsd depends on eps8, so the scheduler cannot sink it)
        eps_sq = consts.tile([GROUPS, 1], f32)
        nc.vector.memset(eps_sq, EPS * EPS)
        eps8 = consts.tile([GROUPS, 1], f32)
        nc.scalar.activation(out=eps8, in_=eps_sq, func=AF.Sqrt, scale=1.0)

        # ---- load x first (chunked, stats interleaved), then consts/weights ----
        xb_sb = []
        stats_l = []
        for ct in range(2):
            cs = slice(ct * 128, (ct + 1) * 128)
            t = big.tile([128, N], bx, name=f"xb_sb{ct}")
            stats = work.tile([128, 8, 6], f32, name="stats", tag="stats")
            for s in range(2):
                fs = slice(s * 2048, (s + 1) * 2048)
                nc.sync.dma_start(out=t[:, fs],
                                  in_=xb_d[cs, fs] if xbf16 else xb_d[cs, fs].bitcast(fr))
                for s2 in range(4):
                    ss = slice(s * 2048 + s2 * 512, s * 2048 + (s2 + 1) * 512)
                    nc.vector.bn_stats(out=stats[:, s * 4 + s2, :],
                                       in_=t[:, ss] if xbf16 else t[:, ss].bitcast(f32))
            xb_sb.append(t)
            stats_l.append(stats)
        xq = []
        for ct in range(2):
            cs = slice(ct * 128, (ct + 1) * 128)
            t = big.tile([128, NQ], f32, name=f"xq_sb{ct}")
            nc.sync.dma_start(out=t, in_=xq_d[cs, :])
            xq.append(t)

        small_sb, wall_sb = [], []
        for ct in range(2):
            cs = slice(ct * 128, (ct + 1) * 128)
            t = consts.tile([128, 5 + GROUPS], f32, name=f"small_sb{ct}")
            nc.sync.dma_start(out=t, in_=small_d[cs, :]); small_sb.append(t)
        gmask8_sb = consts.tile([GROUPS, C], f32)
        nc.sync.dma_start(out=gmask8_sb, in_=gmask8_d)
        for ct in range(2):
            cs = slice(ct * 128, (ct + 1) * 128)
            t = consts.tile([128, 4 * C], f32, name=f"wall_sb{ct}")
            nc.sync.dma_start(out=t, in_=wall_d[cs, :]); wall_sb.append(t)
        gamma_sb = [t[:, 0:1] for t in small_sb]
        beta_sb = [t[:, 1:2] for t in small_sb]
        bq_sb = [t[:, 2:3] for t in small_sb]
        bv_sb = [t[:, 3:4] for t in small_sb]
        bp_sb = [t[:, 4:5] for t in small_sb]
        gmask_sb = [t[:, 5:5 + GROUPS] for t in small_sb]
        wqt_sb = [t[:, 0 * C:1 * C] for t in wall_sb]
        wkt_sb = [t[:, 1 * C:2 * C] for t in wall_sb]
        wvt_sb = [t[:, 2 * C:3 * C] for t in wall_sb]
        wpt_sb = [t[:, 3 * C:4 * C] for t in wall_sb]

        # per-channel moments -> group sums via 0/1 mask matmul (exact fp32)
        gp = pstat.tile([GROUPS, 2], f32, tag="pstat")
        for ct in range(2):
            stile = work.tile([128, 2], f32, name="stile", tag="stile")
            msq = work.tile([128, 1], f32, name="msq", tag="msq")
            nc.vector.bn_aggr(out=stile, in_=stats_l[ct])
            nc.vector.tensor_mul(msq, stile[:, 0:1], stile[:, 0:1])
            nc.vector.tensor_add(stile[:, 1:2], stile[:, 1:2], msq)
            nc.tensor.matmul(gp, lhsT=gmask_sb[ct], rhs=stile,
                             start=(ct == 0), stop=(ct == 1))

        # ---- group stats -> per-channel A, B (PE mask8 broadcast, no DMA) ----
        gms = work.tile([GROUPS, 2], f32, name="gms")
        gvar = work.tile([GROUPS, 1], f32, name="gvar")
        gsd = work.tile([GROUPS, 1], f32, name="gsd")
        gsb = work.tile([GROUPS, 2], f32, name="gsb")
        nc.vector.tensor_scalar_mul(gms, gp, 1.0 / GSIZE)
        nc.vector.tensor_mul(gvar, gms[:, 0:1], gms[:, 0:1])
        nc.vector.tensor_sub(gvar, gms[:, 1:2], gvar)
        nc.scalar.activation(out=gsd, in_=gvar, func=AF.Sqrt, bias=eps8, scale=1.0)
        nc.vector.tensor_copy(gsb[:, 0:1], gms[:, 0:1])
        nc.vector.reciprocal(out=gsb[:, 1:2], in_=gsd)

        A_sb, B_sb = [], []
        for ct in range(2):
            gbp = pstat.tile([128, 2], f32, name="gbp", tag="pstat")
            nc.tensor.matmul(gbp, lhsT=gmask8_sb[:, ct * 128:(ct + 1) * 128],
                             rhs=gsb, start=True, stop=True)
            At = consts.tile([128, 1], f32, name=f"A_sb{ct}")
            Bt = consts.tile([128, 1], f32, name=f"B_sb{ct}")
            nc.vector.tensor_mul(At, gamma_sb[ct], gbp[:, 1:2])
            nc.vector.tensor_mul(Bt, gbp[:, 0:1], At)
            nc.vector.tensor_sub(Bt, beta_sb[ct], Bt)
            A_sb.append(At); B_sb.append(Bt)

        # ---- fold A into weights (split across DVE and ACT) ----
        wqte, wkte, wvte = [], [], []
        for lst, wsrc, nm in ((wkte, wkt_sb, "wkte"), (wqte, wqt_sb, "wqte"),
                              (wvte, wvt_sb, "wvte")):
            for ct in range(2):
                t = consts.tile([128, C], bx, name=f"{nm}{ct}")
                if ct == 0:
                    nc.vector.tensor_scalar_mul(t, wsrc[ct], A_sb[ct])
                else:
                    nc.scalar.activation(out=t, in_=wsrc[ct], func=AF.Identity,
                                         scale=A_sb[ct])
                lst.append(t)
        # preload the Exp act table while PE/DVE are busy with k/q/v
        dummy3 = consts.tile([1, 1], f32)
        nc.scalar.activation(out=dummy3, in_=A_sb[1][0:1, :], func=AF.Exp, scale=1.0)

        # ---- k = (Wk.A) x   [o, m] layout (first PE bulk work) ----
        k_sb = [big.tile([128, N], fr, name=f"k_sb{ot}") for ot in range(2)]
        q_sb = [big.tile([128, NQ], fr, name=f"q_sb{ot}") for ot in range(2)]
        for ot in range(2):
            os_ = slice(ot * 128, (ot + 1) * 128)
            for mc in range(8):
                fs = slice(mc * 512, (mc + 1) * 512)
                kp = pw.tile([128, 512], f32, name="kp", tag="pw")
                for ct in range(2):
                    nc.tensor.matmul(kp, lhsT=R(wkte[ct][:, os_]),
                                     rhs=R(xb_sb[ct][:, fs]),
                                     start=(ct == 0), stop=(ct == 1))
                if mc % 2 == 0:
                    nc.vector.tensor_copy(k_sb[ot][:, fs], kp)
                else:
                    nc.scalar.copy(k_sb[ot][:, fs], kp)

        # ---- bias vectors (PE cost tiny; overlaps with k copies) ----
        def bias_vec(wt_sb, rhs_tiles, badd, nm):
            outs = []
            for oh in range(2):
                p = pstat.tile([128, 1], f32, name=f"{nm}p", tag="pstat")
                for ct in range(2):
                    nc.tensor.matmul(p, lhsT=wt_sb[ct][:, oh * 128:(oh + 1) * 128],
                                     rhs=rhs_tiles[ct], start=(ct == 0), stop=(ct == 1))
                t = consts.tile([128, 1], f32, name=f"{nm}{oh}")
                nc.scalar.activation(out=t, in_=p, func=AF.Identity,
                                     bias=badd[oh], scale=1.0)
                outs.append(t)
            return outs

        cq_sb = bias_vec(wqt_sb, B_sb, bq_sb, "cq")
        cv_sb = bias_vec(wvt_sb, B_sb, bv_sb, "cv")
        bpe_sb = bias_vec(wpt_sb, cv_sb, bp_sb, "bpe")

        # f32r copy of Wp^T for the projection matmuls
        wpte = []
        for ct in range(2):
            t = consts.tile([128, C], fr, name=f"wpte{ct}")
            nc.vector.tensor_copy(t, wpt_sb[ct])
            wpte.append(t)

        # residual+bias base: xqb = x_q + bpe (off critical path)
        xqb = [big.tile([128, NQ], f32, name=f"xqb{ot}") for ot in range(2)]
        for ot in range(2):
            nc.gpsimd.tensor_scalar_add(xqb[ot], xq[ot], bpe_sb[ot])

        # ---- q = (Wq.A) x_q + cq ----
        for ot in range(2):
            os_ = slice(ot * 128, (ot + 1) * 128)
            for qc in range(NCH):
                fs = slice(qc * 512, (qc + 1) * 512)
                qp = pw.tile([128, 512], f32, name="qp", tag="pw")
                for ct in range(2):
                    nc.tensor.matmul(qp, lhsT=R(wqte[ct][:, os_]),
                                     rhs=R(xb_sb[ct][:, fs]),
                                     start=(ct == 0), stop=(ct == 1))
                if ot == 0:
                    nc.scalar.activation(out=q_sb[ot][:, fs], in_=qp,
                                         func=AF.Identity, bias=cq_sb[ot], scale=1.0)
                else:
                    nc.vector.tensor_scalar_add(q_sb[ot][:, fs], qp, cq_sb[ot])

        # ---- v = (Wv.A) x   [m, o] layout ----
        v_sb = big.tile([128, 32, C], fr, name="v_sb")
        for mt in range(32):
            ms = slice(mt * 128, (mt + 1) * 128)
            vp = pw.tile([128, C], f32, name="vp", tag="pw")
            for ct in range(2):
                nc.tensor.matmul(vp, lhsT=R(xb_sb[ct][:, ms]), rhs=R(wvte[ct]),
                                 start=(ct == 0), stop=(ct == 1))
            if mt % 2 == 0:
                nc.scalar.copy(v_sb[:, mt, :], vp)
            else:
                nc.vector.tensor_copy(v_sb[:, mt, :], vp)

        # ---- attention + projection, per n-chunk ----
        att_sb = [big.tile([128, NQ], fr, name=f"att_sb{ot}") for ot in range(2)]
        for nch in range(NCH):
            ns = slice(nch * 512, (nch + 1) * 512)
            otp = [pacc.tile([128, 512], f32, name=f"otp{oh}", tag="acc")
                   for oh in range(2)]
            rp = pr.tile([128, 512], f32, name="rp", tag="pr")
            for mt in range(32):
                ms = slice(mt * 128, (mt + 1) * 128)
                sp = pw.tile([128, 512], f32, name="sp", tag="pw")
                for ot in range(2):
                    nc.tensor.matmul(sp, lhsT=R(k_sb[ot][:, ms]),
                                     rhs=R(q_sb[ot][:, ns]),
                                     start=(ot == 0), stop=(ot == 1))
                e = work.tile([128, 512], fr, name="e", tag="e")
                nc.scalar.activation(out=e, in_=sp, func=AF.Exp, scale=1.0 / 16.0)
                nc.tensor.matmul(rp, lhsT=R(ones128), rhs=R(e),
                                 start=(mt == 0), stop=(mt == 31))
                for oh in range(2):
                    nc.tensor.matmul(otp[oh],
                                     lhsT=R(v_sb[:, mt, oh * 128:(oh + 1) * 128]),
                                     rhs=R(e), start=(mt == 0), stop=(mt == 31))
            rb = work.tile([128, 512], f32, name="rb", tag="rb", bufs=2)
            nc.vector.reciprocal(out=rb, in_=rp)
            # att = (E^T V) * (1/r): normalization folded into the psum drain
            for oh in range(2):
                nc.vector.tensor_mul(att_sb[oh][:, ns], otp[oh], rb)
            # projection + add-only epilogue for this chunk
            last = (nch == NCH - 1)
            st_engines = [nc.sync, nc.scalar] if last else [nc.sync, nc.sync]
            for ot in range(2):
                os_ = slice(ot * 128, (ot + 1) * 128)
                pp = pacc.tile([128, 512], f32, name="pp", tag="acc")
                for ct in range(2):
                    nc.tensor.matmul(pp, lhsT=R(wpte[ct][:, os_]),
                                     rhs=R(att_sb[ct][:, ns]),
                                     start=(ct == 0), stop=(ct == 1))
                ot_t = work.tile([128, 512], f32, name="ot_t", tag="ot_t")
                for hh in range(2):
                    hs = slice(hh * 256, (hh + 1) * 256)
                    ds = slice(nch * 512 + hh * 256, nch * 512 + (hh + 1) * 256)
                    nc.vector.tensor_add(ot_t[:, hs], pp[:, hs], xqb[ot][:, ds])
                st_engines[ot].dma_start(out=out_d[os_, ns], in_=ot_t)

    nc.compile()
    return nc


def _get_nc():
    key = "nc"
    if key not in _CACHE:
        _CACHE[key] = _build_nc(use_f32r=(os.environ.get("BASSK_F32R", "1") == "1"))
    return _CACHE[key]


def _host_inputs(x, gamma, beta, Wq, bq, Wk, bk, Wv, bv, Wp, bp):
    x = np.asarray(x, np.float32)
    xf = np.ascontiguousarray(x.reshape(2, C, N))
    gamma = np.asarray(gamma, np.float32).reshape(C, 1)
    beta = np.asarray(beta, np.float32).reshape(C, 1)
    wqt = np.ascontiguousarray(np.asarray(Wq, np.float32).T)
    wkt = np.ascontiguousarray(np.asarray(Wk, np.float32).T)
    wvt = np.ascontiguousarray(np.asarray(Wv, np.float32).T)
    wpt = np.ascontiguousarray(np.asarray(Wp, np.float32).T)
    bq = np.asarray(bq, np.float32).reshape(C, 1)
    bv = np.asarray(bv, np.float32).reshape(C, 1)
    bp = np.asarray(bp, np.float32).reshape(C, 1)
    gmask = np.zeros((C, GROUPS), np.float32)
    gmask[np.arange(C), np.arange(C) // GSIZE] = 1.0
    gmask8 = np.ascontiguousarray(gmask.T)
    wall = np.ascontiguousarray(np.hstack([wqt, wkt, wvt, wpt]))
    small = np.ascontiguousarray(np.hstack([gamma, beta, bq, bv, bp, gmask]))

    xbf16 = os.environ.get("BASSK_XBF16", "1") == "1"
    if xbf16:
        import ml_dtypes
    in_maps = []
    for core in range(8):
        b, j = divmod(core, 4)
        xrot = np.ascontiguousarray(np.roll(xf[b], -j * NQ, axis=1))
        in_maps.append({
            "xb": xrot.astype(ml_dtypes.bfloat16) if xbf16 else xrot,
            "xq": np.ascontiguousarray(xrot[:, :NQ]),
            "wall": wall, "small": small, "gmask8": gmask8,
        })
    return in_maps


def kernel(x, gamma, beta, Wq, bq, Wk, bk, Wv, bv, Wp, bp):
    from concourse.bass_utils import run_bass_kernel_spmd
    global LAST_RESULTS

    orig_shape = np.asarray(x).shape
    in_maps = _host_inputs(x, gamma, beta, Wq, bq, Wk, bk, Wv, bv, Wp, bp)
    nc = _get_nc()

    trace = os.environ.get("BASSK_TRACE", "0") == "1"
    res = run_bass_kernel_spmd(nc, in_maps, core_ids=list(range(8)), trace=trace)
    LAST_RESULTS = res

    out = np.empty((2, C, N), np.float32)
    for core in range(8):
        b, j = divmod(core, 4)
        out[b][:, j * NQ:(j + 1) * NQ] = res.results[core]["out"]
    return out.reshape(orig_shape)



# revision 17
# speedup vs baseline: 1.9206x; 1.9206x over previous
"""AttentionBlock3D (GroupNorm + single-head self-attention + residual) on 8 TRN2 cores.

Sharding: core = (batch b in {0,1}) x (1024-row slice of the 4096 attention rows).
Each core computes its batch's GroupNorm stats (cheap, on the PE) and
attention + output projection for its own 1024 query rows. No collectives.
The host ROTATES each core's x copy so that its query rows are always
columns 0..1024 (attention is permutation-invariant over keys).

fp8 DoubleRow pipeline with K and V eliminated:
  - x ships twice in fp8 e4m3: xb8 [128, 2, N] (channel-major) and
    xt8 [128, 32, 258] (position-major with built-in ones columns).
  - GroupNorm stats on the PE: x^T x accumulated per channel half; the ones
    column gives Sum x, the diagonal (mask reduce) gives Sum x^2.
  - S = hn^T G hn_q with G = Wk^T Wq (host): q' = (G.A-folded)^T x_q scaled
    by A, S = x^T q' via DoubleRow with xb8 stationary — no K tensor.
    Per-key bias delta[m] = (A o (G B + Wk^T bq))^T x[:,m] enters S as one
    extra DoubleRow matmul with constant-column rhs (coef broadcast).
  - u = x^T E via DoubleRow with xt8 stationary — no V tensor; the output
    projection fuses Pv = Wp @ Wv (host) with A folded on device:
    out = x_q + bpe + (Pv.A)^T u / r,  bpe = bp + Wp cv, cv = Wv B + bv.
  - E = exp(S/16 - 2) on ACT in [128,1024] tiles (e^-2 cancels against r).
    ACT exp is the bottleneck engine (~4.2M elements/core).
"""

import os
import numpy as np
from contextlib import ExitStack

C = 256          # channels
N = 4096         # spatial positions (16*16*16)
NQ = 1024        # query rows per core
GROUPS = 8
GSIZE = C // GROUPS
EPS = 1e-5

_CACHE = {}
LAST_RESULTS = None  # test harness can inspect trace results


def _build_nc():
    import concourse.bacc as bacc
    import concourse.tile as tile
    from concourse import mybir

    f32 = mybir.dt.float32
    f32r = mybir.dt.float32r
    bf16 = mybir.dt.bfloat16
    f8 = mybir.dt.float8e4
    AF = mybir.ActivationFunctionType
    DR = mybir.MatmulPerfMode.DoubleRow
    ADD = mybir.AluOpType.add
    MULT = mybir.AluOpType.mult

    nc = bacc.Bacc("TRN2", target_bir_lowering=False, debug=False,
                   enable_asserts=False)

    # ---- DRAM I/O (per-core) ----
    xt8_d = nc.dram_tensor("xt8", [128, 32 * 272], f8, kind="ExternalInput").ap()
    xb8_d = nc.dram_tensor("xb8", [128, 2 * N], f8, kind="ExternalInput").ap()
    gt_d = nc.dram_tensor("gt", [128, 2 * C], bf16, kind="ExternalInput").ap()
    pvt_d = nc.dram_tensor("pvt", [128, 2 * C], bf16, kind="ExternalInput").ap()
    wall_d = nc.dram_tensor("wall", [128, 2 * 2 * C], bf16, kind="ExternalInput").ap()
    small_d = nc.dram_tensor("small", [128, 24], f32, kind="ExternalInput").ap()
    gmask8_d = nc.dram_tensor("gmask8", [GROUPS, C], f32, kind="ExternalInput").ap()
    imask_d = nc.dram_tensor("imask", [128, 128], f32, kind="ExternalInput").ap()
    bprow_d = nc.dram_tensor("bprow", [1, C], f32, kind="ExternalInput").ap()
    xq_d = nc.dram_tensor("xq", [128, 2 * NQ], f32, kind="ExternalInput").ap()
    out_d = nc.dram_tensor("out", [128, 2 * NQ], f32, kind="ExternalOutput").ap()
    DBG = os.environ.get("BASSK_DBG", "0") == "1"
    if DBG:
        dbg_stile = nc.dram_tensor("dbg_stile", [128, 4], f32, kind="ExternalOutput").ap()
        dbg_a2 = nc.dram_tensor("dbg_a2", [128, 4], f32, kind="ExternalOutput").ap()
        dbg_q8 = nc.dram_tensor("dbg_q8", [128, 2 * NQ], f32, kind="ExternalOutput").ap()
        dbg_coef = nc.dram_tensor("dbg_coef", [128, 2 * 512], f32, kind="ExternalOutput").ap()
        dbg_sp = nc.dram_tensor("dbg_sp", [128, 1024], f32, kind="ExternalOutput").ap()
        dbg_rp = nc.dram_tensor("dbg_rp", [128, 512], f32, kind="ExternalOutput").ap()
        dbg_u8 = nc.dram_tensor("dbg_u8", [128, 2 * NQ], f32, kind="ExternalOutput").ap()
        dbg_bpe = nc.dram_tensor("dbg_bpe", [1, C], f32, kind="ExternalOutput").ap()

    with tile.TileContext(nc) as tc, ExitStack() as ctx:
        big = ctx.enter_context(tc.tile_pool(name="big", bufs=1))
        consts = ctx.enter_context(tc.tile_pool(name="consts", bufs=1))
        work = ctx.enter_context(tc.tile_pool(name="work", bufs=3))
        epool = ctx.enter_context(tc.tile_pool(name="epool", bufs=3))
        # PSUM banks: psp 2x2 + pacc 2 + pr 1 + pw 1 = 8
        psp = ctx.enter_context(tc.tile_pool(name="psp", bufs=2, space="PSUM"))
        pacc = ctx.enter_context(tc.tile_pool(name="pacc", bufs=2, space="PSUM"))
        pr = ctx.enter_context(tc.tile_pool(name="pr", bufs=1, space="PSUM"))
        pw = ctx.enter_context(tc.tile_pool(name="pw", bufs=1, space="PSUM"))

        # ---- constants (before the big loads) ----
        ones8 = consts.tile([128, 2, 128], f8)
        nc.vector.memset(ones8, 1.0)
        nbias = consts.tile([128, 1], f32)
        nc.vector.memset(nbias, -3.5)
        zero512 = consts.tile([128, 512], f32)
        nc.vector.memset(zero512, 0.0)
        # eps8 = Sqrt(EPS^2) on ACT: forces the Sqrt act-table load at t~0
        eps_sq = consts.tile([GROUPS, 1], f32)
        nc.vector.memset(eps_sq, EPS * EPS)
        eps8 = consts.tile([GROUPS, 1], f32)
        nc.scalar.activation(out=eps8, in_=eps_sq, func=AF.Sqrt, scale=1.0)
        # preload the Exp act table right after (ACT idle during the head)
        dummye = consts.tile([1, 1], f32)
        nc.scalar.activation(out=dummye, in_=eps_sq[0:1, :], func=AF.Exp,
                             scale=1.0, bias=nbias[0:1, :])

        # ---- load xt8 (stats + u path) chunked; stats matmuls interleave ----
        xt8 = big.tile([128, 32, 272], f8)
        for ch in range(4):
            nc.sync.dma_start(out=xt8[:, 8 * ch:8 * ch + 8, :],
                              in_=xt8_d[:, 8 * 272 * ch:8 * 272 * (ch + 1)])
        # xx[h] accumulates x^T x for channel half h ([128,129]: 128 cols of
        # the x^T x block + ones column giving Sum x). Uses the idle sp slots.
        xx = []
        for h in range(2):
            t = psp.tile([128, 129], f32, name=f"xx{h}", tag="sp")
            xx.append(t)
        for s in range(16):
            for h in range(2):
                nc.tensor.matmul(
                    xx[h],
                    lhsT=xt8[:, 2 * s:2 * s + 2, 136 * h:136 * h + 128],
                    rhs=xt8[:, 2 * s:2 * s + 2, 136 * h:136 * h + 129],
                    start=(s == 0), stop=(s == 15), perf_mode=DR)

        # ---- smalls + G^T + xb8 query cols (early, for q') ----
        small_sb = consts.tile([128, 24], f32)
        nc.sync.dma_start(out=small_sb, in_=small_d)
        gamma2 = small_sb[:, 0:2]
        beta2 = small_sb[:, 2:4]
        bv2 = small_sb[:, 4:6]
        wtld2 = small_sb[:, 6:8]          # Wk^T bq (host)
        gmaskT = [small_sb[:, 8 + 8 * i:16 + 8 * i] for i in range(2)]
        imask = consts.tile([128, 128], f32)
        nc.sync.dma_start(out=imask, in_=imask_d)
        gmask8 = consts.tile([GROUPS, C], f32)
        nc.sync.dma_start(out=gmask8, in_=gmask8_d)
        gt = consts.tile([128, 2, C], bf16)
        nc.sync.dma_start(out=gt, in_=gt_d)
        xb8 = big.tile([128, 2, N], f8)
        for i in range(2):
            nc.sync.dma_start(out=xb8[:, i, 0:NQ], in_=xb8_d[:, N * i:N * i + NQ])

        # ---- group stats -> per-channel A (f32) and B (bf16) ----
        # Sum x^2 = diag(x^T x): mask out the diagonal, then column-sum it
        # back to [128,1] with a N=1 matmul (diag matrix -> col sums = diag).
        ones_col = consts.tile([128, 2], f32)
        nc.vector.memset(ones_col, 1.0)
        ones_colr = consts.tile([128, 2], f32r)
        nc.vector.tensor_copy(ones_colr, ones_col)
        stile = work.tile([128, 2, 2], f32, name="stile")  # [:, h, (sx, sxx)]
        for h in range(2):
            nc.vector.tensor_copy(stile[:, h, 0:1], xx[h][:, 128:129])
            scr = work.tile([128, 128], f32r, name="scr", tag="scr", bufs=2)
            nc.vector.tensor_mul(scr, xx[h][:, 0:128], imask)
            sxp = pw.tile([128, 2], f32, name="sxp", tag="pw")
            nc.tensor.matmul(sxp, lhsT=scr, rhs=ones_colr, start=True, stop=True)
            nc.vector.tensor_copy(stile[:, h, 1:2], sxp[:, 0:1])
        gp = pw.tile([GROUPS, 2], f32, tag="pw")
        for h in range(2):
            nc.tensor.matmul(gp, lhsT=gmaskT[h], rhs=stile[:, h, :],
                             start=(h == 0), stop=(h == 1))
        gms = work.tile([GROUPS, 2], f32, name="gms")
        gvar = work.tile([GROUPS, 1], f32, name="gvar")
        gsd = work.tile([GROUPS, 1], f32, name="gsd")
        gsb = work.tile([GROUPS, 2], f32, name="gsb")
        nc.vector.tensor_scalar_mul(gms, gp, 1.0 / (GSIZE * N))
        nc.vector.tensor_mul(gvar, gms[:, 0:1], gms[:, 0:1])
        nc.vector.tensor_sub(gvar, gms[:, 1:2], gvar)
        nc.scalar.activation(out=gsd, in_=gvar, func=AF.Sqrt, bias=eps8, scale=1.0)
        nc.vector.tensor_copy(gsb[:, 0:1], gms[:, 0:1])
        nc.vector.reciprocal(out=gsb[:, 1:2], in_=gsd)

        A2 = consts.tile([128, 2], f32)
        B2 = consts.tile([128, 2], bf16)
        B2f = work.tile([128, 2], f32, name="B2f")
        for i in range(2):
            gbp = pw.tile([128, 2], f32, name="gbp", tag="pw")
            nc.tensor.matmul(gbp, lhsT=gmask8[:, 128 * i:128 * (i + 1)],
                             rhs=gsb, start=True, stop=True)
            nc.vector.tensor_mul(A2[:, i:i + 1], gamma2[:, i:i + 1], gbp[:, 1:2])
            nc.vector.tensor_mul(B2f[:, i:i + 1], gbp[:, 0:1], A2[:, i:i + 1])
            nc.vector.tensor_sub(B2f[:, i:i + 1], beta2[:, i:i + 1],
                                 B2f[:, i:i + 1])
        nc.vector.tensor_copy(B2, B2f)
        if DBG:
            dt1 = work.tile([128, 4], f32, name="dt1")
            nc.vector.tensor_copy(dt1, stile)
            nc.sync.dma_start(out=dbg_stile, in_=dt1)
            dt2 = work.tile([128, 4], f32, name="dt2")
            nc.vector.tensor_copy(dt2[:, 0:2], A2)
            nc.vector.tensor_copy(dt2[:, 2:4], B2f)
            nc.sync.dma_start(out=dbg_a2, in_=dt2)

        # ---- fold A into G^T -> fp8 (contraction-side fold for q') ----
        ga8 = consts.tile([128, 2, C], f8)
        for i in range(2):
            nc.vector.tensor_scalar_mul(ga8[:, i, :], gt[:, i, :], A2[:, i:i + 1])

        # ---- q'[c, n] = A[c] * sum_c' (G[c,c'] A[c']) x_q[c', n] ----
        q8 = big.tile([128, 2, NQ], f8)
        for ch in range(2):
            for qc in range(2):
                ns = slice(qc * 512, (qc + 1) * 512)
                qp = psp.tile([128, 512], f32, name="qp", tag="sp")
                nc.tensor.matmul(qp, lhsT=ga8[:, :, ch * 128:(ch + 1) * 128],
                                 rhs=xb8[:, :, ns], start=True, stop=True,
                                 perf_mode=DR)
                nc.vector.tensor_scalar_mul(q8[:, ch, ns], qp, A2[:, ch:ch + 1])

        # ---- delta coefficient: coef = A o (G B + Wk^T bq), broadcast fp8 ----
        gbv = pw.tile([128, 2], f32, name="gbv", tag="pw")
        for ch in range(2):
            for i in range(2):
                nc.tensor.matmul(gbv[:, ch:ch + 1],
                                 lhsT=gt[:, i, ch * 128:(ch + 1) * 128],
                                 rhs=B2[:, i:i + 1], start=(i == 0), stop=(i == 1))
        coef = work.tile([128, 2], f32, name="coef")
        nc.vector.tensor_add(coef, gbv, wtld2)
        nc.vector.tensor_mul(coef, coef, A2)
        coef8b = consts.tile([128, 2, 512], f8)
        for i in range(2):
            nc.vector.tensor_scalar_add(coef8b[:, i, :], zero512,
                                        coef[:, i:i + 1])

        if DBG:
            dt3 = work.tile([128, 2, NQ], f32, name="dt3", tag="dbgbig")
            nc.vector.tensor_copy(dt3, q8)
            nc.sync.dma_start(out=dbg_q8, in_=dt3)
            dt4 = work.tile([128, 2, 512], f32, name="dt4", tag="dbgbig2")
            nc.vector.tensor_copy(dt4, coef8b)
            nc.sync.dma_start(out=dbg_coef, in_=dt4)

        # ---- rest of the loads (xb8 keys, weights for biases, xq last) ----
        for i in range(2):
            nc.sync.dma_start(out=xb8[:, i, NQ:N],
                              in_=xb8_d[:, N * i + NQ:N * (i + 1)])
        wall = consts.tile([128, 2, 2 * C], bf16)   # [wv, wp]
        nc.sync.dma_start(out=wall, in_=wall_d)
        pvt = consts.tile([128, 2, C], bf16)
        nc.sync.dma_start(out=pvt, in_=pvt_d)
        bprow = consts.tile([1, C], f32)
        nc.sync.dma_start(out=bprow, in_=bprow_d)
        xq = big.tile([128, 2, NQ], f32)
        nc.sync.dma_start(out=xq, in_=xq_d)

        # ---- fold A into Pv^T -> fp8 (fused Wp @ Wv projection) ----
        pva8 = consts.tile([128, 2, C], f8)
        for i in range(2):
            nc.vector.tensor_scalar_mul(pva8[:, i, :], pvt[:, i, :],
                                        A2[:, i:i + 1])

        # ---- cv = Wv B + bv (bf16), bpeT [1, C] = bp + Wp cv ----
        cvf = work.tile([128, 2], f32, name="cvf")
        for ot in range(2):
            p = pw.tile([128, 1], f32, name="cvp", tag="pw")
            for i in range(2):
                nc.tensor.matmul(p, lhsT=wall[:, i, 128 * ot:128 * (ot + 1)],
                                 rhs=B2[:, i:i + 1], start=(i == 0), stop=(i == 1))
            nc.vector.tensor_scalar_add(cvf[:, ot:ot + 1], p, bv2[:, ot:ot + 1])
        cv = consts.tile([128, 2], bf16)
        nc.vector.tensor_copy(cv, cvf)
        bpeT_p = pw.tile([1, C], f32, tag="pw")
        for i in range(2):
            nc.tensor.matmul(bpeT_p, lhsT=cv[:, i:i + 1],
                             rhs=wall[:, i, C:2 * C], start=(i == 0), stop=(i == 1))
        bpeT_f = work.tile([1, C], f32, name="bpeT_f")
        nc.vector.tensor_add(bpeT_f, bpeT_p, bprow)
        bpeT = consts.tile([1, C], f32r)
        nc.vector.tensor_copy(bpeT, bpeT_f)
        ones_row = consts.tile([1, 512], f32)
        nc.vector.memset(ones_row, 1.0)
        ones_rowr = consts.tile([1, 512], f32r)
        nc.vector.tensor_copy(ones_rowr, ones_row)

        # ---- attention: S + delta -> exp -> r, u0, u1 -> scale -> project ----
        u8 = big.tile([128, 2, NQ], f8)
        for nch in range(2):
            ns = slice(nch * 512, (nch + 1) * 512)
            rp = pr.tile([128, 512], f32, name="rp", tag="r")
            up = [pacc.tile([128, 512], f32, name=f"up{h}", tag="u")
                  for h in range(2)]
            for s in range(16):
                sp = psp.tile([128, 1024], f32, name="sp", tag="sp")
                for h in range(2):
                    ms = slice((2 * s + h) * 128, (2 * s + h + 1) * 128)
                    hs = slice(512 * h, 512 * (h + 1))
                    nc.tensor.matmul(sp[:, hs], lhsT=xb8[:, :, ms],
                                     rhs=q8[:, :, ns], start=True, stop=False,
                                     perf_mode=DR)
                    nc.tensor.matmul(sp[:, hs], lhsT=xb8[:, :, ms],
                                     rhs=coef8b, start=False, stop=True,
                                     perf_mode=DR)
                e = epool.tile([128, 2, 512], f8, name="e", tag="e")
                if DBG and nch == 0 and s == 0:
                    dt5 = work.tile([128, 1024], f32, name="dt5", tag="dbgbig3")
                    nc.vector.tensor_copy(dt5, sp)
                    nc.sync.dma_start(out=dbg_sp, in_=dt5)
                nc.scalar.activation(out=e, in_=sp, func=AF.Exp,
                                     scale=1.0 / 16.0, bias=nbias)
                nc.tensor.matmul(rp, lhsT=ones8, rhs=e,
                                 start=(s == 0), stop=(s == 15), perf_mode=DR)
                for h in range(2):
                    nc.tensor.matmul(
                        up[h],
                        lhsT=xt8[:, 2 * s:2 * s + 2, 136 * h:136 * h + 128],
                        rhs=e, start=(s == 0), stop=(s == 15), perf_mode=DR)
            if DBG and nch == 0:
                dt6 = work.tile([128, 512], f32, name="dt6", tag="dbgbig4")
                nc.vector.tensor_copy(dt6, rp)
                nc.sync.dma_start(out=dbg_rp, in_=dt6)
            rb = work.tile([128, 512], f32, name="rb", tag="rb", bufs=2)
            nc.vector.reciprocal(out=rb, in_=rp)
            for h in range(2):
                nc.vector.tensor_mul(u8[:, h, ns], up[h], rb)

            if DBG and nch == 1:
                dt7 = work.tile([128, 2, NQ], f32, name="dt7", tag="dbgbig5")
                nc.vector.tensor_copy(dt7, u8)
                nc.sync.dma_start(out=dbg_u8, in_=dt7)
                dt8 = work.tile([1, C], f32, name="dt8", tag="dbgbig6")
                nc.vector.tensor_copy(dt8, bpeT)
                nc.sync.dma_start(out=dbg_bpe, in_=dt8)
            # fused projection + bpe + residual epilogue
            for ot in range(2):
                pp = pw.tile([128, 512], f32, name="pp", tag="pw")
                nc.tensor.matmul(pp, lhsT=pva8[:, :, ot * 128:(ot + 1) * 128],
                                 rhs=u8[:, :, ns], start=True, stop=False,
                                 perf_mode=DR)
                nc.tensor.matmul(pp, lhsT=bpeT[:, ot * 128:(ot + 1) * 128],
                                 rhs=ones_rowr, start=False, stop=True)
                ot_t = work.tile([128, 512], f32, name="ot_t", tag="ot_t")
                nc.vector.tensor_add(ot_t, pp, xq[:, ot, ns])
                nc.sync.dma_start(out=out_d[:, NQ * ot + 512 * nch:
                                            NQ * ot + 512 * (nch + 1)],
                                  in_=ot_t)

    nc.compile()
    return nc


def _get_nc():
    key = "nc"
    if key not in _CACHE:
        _CACHE[key] = _build_nc()
    return _CACHE[key]


def _host_inputs(x, gamma, beta, Wq, bq, Wk, bk, Wv, bv, Wp, bp):
    import ml_dtypes
    f8 = ml_dtypes.float8_e4m3
    bf = ml_dtypes.bfloat16

    x = np.asarray(x, np.float32)
    xf = np.ascontiguousarray(x.reshape(2, C, N))
    gamma = np.asarray(gamma, np.float32)
    beta = np.asarray(beta, np.float32)
    Wq, Wk, Wv, Wp = [np.asarray(W, np.float32) for W in (Wq, Wk, Wv, Wp)]
    bq, bv, bp = [np.asarray(v, np.float32) for v in (bq, bv, bp)]

    # host-fused matrices: G = Wk^T Wq, Pv = Wp @ Wv
    G = Wk.T @ Wq                                             # [c, c']
    Pv = Wp @ Wv                                              # [o, c]
    # gt[p, i, c] = G[c, i*128+p]  (contraction over c' = i*128+p)
    gt = np.ascontiguousarray(
        G.T.reshape(2, 128, C).transpose(1, 0, 2).reshape(128, 2 * C)
    ).astype(bf)
    # pvt[p, i, o] = Pv[o, i*128+p]
    pvt = np.ascontiguousarray(
        Pv.T.reshape(2, 128, C).transpose(1, 0, 2).reshape(128, 2 * C)
    ).astype(bf)
    # wall[p, i, (wv|wp), o] = W[o, i*128+p]
    wall = np.stack([Wv.T, Wp.T], axis=1)                     # [c, 2, o]
    wall = wall.reshape(2, 128, 2, C).transpose(1, 0, 2, 3)
    wall = np.ascontiguousarray(wall.reshape(128, 2 * 2 * C)).astype(bf)

    small = np.zeros((128, 24), np.float32)
    small[:, 0:2] = gamma.reshape(2, 128).T
    small[:, 2:4] = beta.reshape(2, 128).T
    small[:, 4:6] = bv.reshape(2, 128).T
    small[:, 6:8] = (Wk.T @ bq).reshape(2, 128).T
    cids = np.arange(C)
    gm = np.zeros((C, GROUPS), np.float32)
    gm[cids, cids // GSIZE] = 1.0                             # [c, g]
    gmT = gm.reshape(2, 128, GROUPS).transpose(1, 0, 2)       # [p, i, g]
    small[:, 8:16] = gmT[:, 0, :]
    small[:, 16:24] = gmT[:, 1, :]
    gmask8 = np.ascontiguousarray(gm.T)                       # [g, c]
    imask = np.eye(128, dtype=np.float32)
    bprow = np.ascontiguousarray(bp.reshape(1, C))

    in_maps = []
    for core in range(8):
        b, j = divmod(core, 4)
        xrot = np.roll(xf[b], -j * NQ, axis=1)                # [C, N]
        x8 = xrot.astype(f8)
        xb8 = np.ascontiguousarray(
            x8.reshape(2, 128, N).transpose(1, 0, 2).reshape(128, 2 * N))
        xqv = np.ascontiguousarray(
            xrot[:, :NQ].reshape(2, 128, NQ).transpose(1, 0, 2)
            .reshape(128, 2 * NQ))
        # xt8: position-major with ones cols: per n: [c0..c127, 1, c128.., 1]
        xt = x8.astype(np.float32).T                          # [n, c] quantized
        arr = np.zeros((N, 272), np.float32)
        arr[:, 0:128] = xt[:, 0:128]
        arr[:, 128] = 1.0
        arr[:, 136:264] = xt[:, 128:256]
        arr[:, 264] = 1.0
        xt8 = np.ascontiguousarray(
            arr.reshape(32, 128, 272).transpose(1, 0, 2).reshape(128, 32 * 272)
        ).astype(f8)
        in_maps.append({
            "xt8": xt8, "xb8": xb8, "gt": gt, "pvt": pvt, "wall": wall,
            "small": small, "gmask8": gmask8, "imask": imask,
            "bprow": bprow, "xq": xqv,
        })
    return in_maps


def kernel(x, gamma, beta, Wq, bq, Wk, bk, Wv, bv, Wp, bp):
    from concourse.bass_utils import run_bass_kernel_spmd
    global LAST_RESULTS

    orig_shape = np.asarray(x).shape
    in_maps = _host_inputs(x, gamma, beta, Wq, bq, Wk, bk, Wv, bv, Wp, bp)
    nc = _get_nc()

    trace = os.environ.get("BASSK_TRACE", "0") == "1"
    res = run_bass_kernel_spmd(nc, in_maps, core_ids=list(range(8)), trace=trace)
    LAST_RESULTS = res

    out = np.empty((2, C, N), np.float32)
    for core in range(8):
        b, j = divmod(core, 4)
        o = res.results[core]["out"]                          # [128, 2*NQ]
        o = o.reshape(128, 2, NQ).transpose(1, 0, 2).reshape(C, NQ)
        out[b][:, j * NQ:(j + 1) * NQ] = o
    return out.reshape(orig_shape)


# revision 23
# speedup vs baseline: 2.0729x; 1.0793x over previous
"""AttentionBlock3D (GroupNorm + single-head self-attention + residual) on 8 TRN2 cores.

Sharding: core = (batch b in {0,1}) x (1024-row slice of the 4096 attention rows).
Each core computes its batch's GroupNorm stats (cheap, on the PE) and
attention + output projection for its own 1024 query rows. No collectives.
The host ROTATES each core's x copy so that its query rows are always
columns 0..1024 (attention is permutation-invariant over keys).

fp8 DoubleRow pipeline with K and V eliminated:
  - x ships twice in fp8 e4m3: xb8 [128, 2, N] (channel-major) and
    xt8 [128, 32, 258] (position-major with built-in ones columns).
  - GroupNorm stats on the PE: x^T x accumulated per channel half; the ones
    column gives Sum x, the diagonal (mask reduce) gives Sum x^2.
  - S = hn^T G hn_q with G = Wk^T Wq (host): q' = (G.A-folded)^T x_q scaled
    by A, S = x^T q' via DoubleRow with xb8 stationary — no K tensor.
    Per-key bias delta[m] = (A o (G B + Wk^T bq))^T x[:,m] enters S as one
    extra DoubleRow matmul with constant-column rhs (coef broadcast).
  - u = x^T E via DoubleRow with xt8 stationary — no V tensor; the output
    projection fuses Pv = Wp @ Wv (host) with A folded on device:
    out = x_q + bpe + (Pv.A)^T u / r,  bpe = bp + Wp cv, cv = Wv B + bv.
  - E = exp(S/16 - 2) on ACT in [128,1024] tiles (e^-2 cancels against r).
    ACT exp is the bottleneck engine (~4.2M elements/core).
"""

import os
import numpy as np
from contextlib import ExitStack

C = 256          # channels
N = 4096         # spatial positions (16*16*16)
NQ = 1024        # query rows per core
GROUPS = 8
GSIZE = C // GROUPS
EPS = 1e-5

_CACHE = {}
LAST_RESULTS = None  # test harness can inspect trace results


def _build_nc():
    import concourse.bacc as bacc
    import concourse.tile as tile
    from concourse import mybir

    f32 = mybir.dt.float32
    f32r = mybir.dt.float32r
    bf16 = mybir.dt.bfloat16
    f8 = mybir.dt.float8e4
    AF = mybir.ActivationFunctionType
    DR = mybir.MatmulPerfMode.DoubleRow
    ADD = mybir.AluOpType.add
    MULT = mybir.AluOpType.mult

    nc = bacc.Bacc("TRN2", target_bir_lowering=False, debug=False,
                   enable_asserts=False)

    # ---- DRAM I/O (per-core) ----
    xt8_d = nc.dram_tensor("xt8", [128, 32 * 272], f8, kind="ExternalInput").ap()
    xb8_d = nc.dram_tensor("xb8", [128, 2 * N], f8, kind="ExternalInput").ap()
    gt_d = nc.dram_tensor("gt", [128, 2 * C], bf16, kind="ExternalInput").ap()
    pvt_d = nc.dram_tensor("pvt", [128, 2 * C], bf16, kind="ExternalInput").ap()
    wall_d = nc.dram_tensor("wall", [128, 2 * 2 * C], bf16, kind="ExternalInput").ap()
    small_d = nc.dram_tensor("small", [128, 26], f32, kind="ExternalInput").ap()
    gmask8_d = nc.dram_tensor("gmask8", [GROUPS, C], f32, kind="ExternalInput").ap()
    imask_d = nc.dram_tensor("imask", [128, 128], f32, kind="ExternalInput").ap()
    xq_d = nc.dram_tensor("xq", [128, 2 * NQ], f32, kind="ExternalInput").ap()
    out_d = nc.dram_tensor("out", [128, 2 * NQ], f32, kind="ExternalOutput").ap()
    DBG = os.environ.get("BASSK_DBG", "0") == "1"
    if DBG:
        dbg_stile = nc.dram_tensor("dbg_stile", [128, 4], f32, kind="ExternalOutput").ap()
        dbg_a2 = nc.dram_tensor("dbg_a2", [128, 4], f32, kind="ExternalOutput").ap()
        dbg_q8 = nc.dram_tensor("dbg_q8", [128, 2 * NQ], f32, kind="ExternalOutput").ap()
        dbg_coef = nc.dram_tensor("dbg_coef", [128, 2 * 512], f32, kind="ExternalOutput").ap()
        dbg_sp = nc.dram_tensor("dbg_sp", [128, 1024], f32, kind="ExternalOutput").ap()
        dbg_rp = nc.dram_tensor("dbg_rp", [128, 512], f32, kind="ExternalOutput").ap()
        dbg_u8 = nc.dram_tensor("dbg_u8", [128, 2 * NQ], f32, kind="ExternalOutput").ap()
        dbg_bpe = nc.dram_tensor("dbg_bpe", [1, C], f32, kind="ExternalOutput").ap()

    with tile.TileContext(nc) as tc, ExitStack() as ctx:
        big = ctx.enter_context(tc.tile_pool(name="big", bufs=1))
        consts = ctx.enter_context(tc.tile_pool(name="consts", bufs=1))
        work = ctx.enter_context(tc.tile_pool(name="work", bufs=3))
        epool = ctx.enter_context(tc.tile_pool(name="epool", bufs=3))
        # PSUM banks: psp 2x2 + pacc 2 + pr 1 + pw 1 = 8
        psp = ctx.enter_context(tc.tile_pool(name="psp", bufs=2, space="PSUM"))
        pacc = ctx.enter_context(tc.tile_pool(name="pacc", bufs=2, space="PSUM"))
        pr = ctx.enter_context(tc.tile_pool(name="pr", bufs=1, space="PSUM"))
        pw = ctx.enter_context(tc.tile_pool(name="pw", bufs=1, space="PSUM"))

        # ---- constants (before the big loads) ----
        # r-matmul stationary = 1/64 so the reciprocal yields 64/r; the /64
        # is repaid in the epilogue stt. Keeps u8 = 64*u/r in fp8's sweet spot.
        ones8 = consts.tile([128, 2, 128], f8)
        nc.vector.memset(ones8, 1.0 / 64.0)
        nbias = consts.tile([128, 1], f32)
        nc.vector.memset(nbias, -3.5)
        # eps8 = Sqrt(EPS^2) on ACT: forces the Sqrt act-table load at t~0
        eps_sq = consts.tile([GROUPS, 1], f32)
        nc.vector.memset(eps_sq, EPS * EPS)
        eps8 = consts.tile([GROUPS, 1], f32)
        nc.scalar.activation(out=eps8, in_=eps_sq, func=AF.Sqrt, scale=1.0)

        # ---- load xt8 (stats + u path) chunked; stats matmuls interleave ----
        xt8 = big.tile([128, 32, 272], f8)
        for ch in range(4):
            nc.sync.dma_start(out=xt8[:, 8 * ch:8 * ch + 8, :],
                              in_=xt8_d[:, 8 * 272 * ch:8 * 272 * (ch + 1)])
        # xx[h] accumulates x^T x for channel half h ([128,129]: 128 cols of
        # the x^T x block + ones column giving Sum x). Uses the idle sp slots.
        xx = []
        for h in range(2):
            t = psp.tile([128, 129], f32, name=f"xx{h}", tag="sp")
            xx.append(t)
        for s in range(16):
            for h in range(2):
                nc.tensor.matmul(
                    xx[h],
                    lhsT=xt8[:, 2 * s:2 * s + 2, 136 * h:136 * h + 128],
                    rhs=xt8[:, 2 * s:2 * s + 2, 136 * h:136 * h + 129],
                    start=(s == 0), stop=(s == 15), perf_mode=DR)

        # ---- smalls + G^T + xb8 query cols (early, for q') ----
        small_sb = consts.tile([128, 26], f32)
        nc.sync.dma_start(out=small_sb, in_=small_d)
        gamma2 = small_sb[:, 0:2]
        beta2 = small_sb[:, 2:4]
        bv2 = small_sb[:, 4:6]
        wtld2 = small_sb[:, 6:8]          # Wk^T bq (host)
        bp2 = small_sb[:, 8:10]
        gmaskT = [small_sb[:, 10 + 8 * i:18 + 8 * i] for i in range(2)]  # pre-scaled 1/(32N)
        imask = consts.tile([128, 128], f32)
        nc.sync.dma_start(out=imask, in_=imask_d)
        gmask8 = consts.tile([GROUPS, C], f32)
        nc.sync.dma_start(out=gmask8, in_=gmask8_d)
        gt = consts.tile([128, 2, C], bf16)
        nc.sync.dma_start(out=gt, in_=gt_d)
        xb8 = big.tile([128, 2, N], f8)
        for i in range(2):
            nc.sync.dma_start(out=xb8[:, i, 0:NQ], in_=xb8_d[:, N * i:N * i + NQ])

        # ---- group stats -> per-channel A (f32) and B (bf16) ----
        # Sum x^2 = diag(x^T x): mask out the diagonal, then column-sum it
        # back to [128,1] with a N=1 matmul (diag matrix -> col sums = diag).
        ones_col = consts.tile([128, 2], f32)
        nc.vector.memset(ones_col, 1.0)
        ones_colr = consts.tile([128, 2], f32r)
        nc.vector.tensor_copy(ones_colr, ones_col)
        stile = work.tile([128, 2, 2], f32, name="stile")  # [:, h, (sx, sxx)]
        for h in range(2):
            nc.vector.tensor_copy(stile[:, h, 0:1], xx[h][:, 128:129])
            scr = work.tile([128, 128], f32r, name="scr", tag="scr", bufs=2)
            nc.vector.tensor_mul(scr, xx[h][:, 0:128], imask)
            sxp = pw.tile([128, 2], f32, name="sxp", tag="pw")
            nc.tensor.matmul(sxp, lhsT=scr, rhs=ones_colr, start=True, stop=True)
            nc.vector.tensor_copy(stile[:, h, 1:2], sxp[:, 0:1])
        gp = pw.tile([GROUPS, 2], f32, tag="pw")
        for h in range(2):
            nc.tensor.matmul(gp, lhsT=gmaskT[h], rhs=stile[:, h, :],
                             start=(h == 0), stop=(h == 1))
        # gmaskT is host-scaled by 1/(32N), so gp = (mean, E[x^2]) directly.
        gsb = work.tile([GROUPS, 2], f32, name="gsb")
        nc.vector.tensor_copy(gsb, gp)
        negvar = work.tile([GROUPS, 1], f32, name="negvar")
        nc.vector.scalar_tensor_tensor(out=negvar, in0=gsb[:, 0:1],
                                       scalar=gsb[:, 0:1], in1=gsb[:, 1:2],
                                       op0=MULT, op1=mybir.AluOpType.subtract)
        gsd = work.tile([GROUPS, 1], f32, name="gsd")
        nc.scalar.activation(out=gsd, in_=negvar, func=AF.Sqrt, bias=eps8,
                             scale=-1.0)
        # preload the Exp act table now (the Sqrt above was ACT's last
        # non-Exp op; loading here keeps the main loop table-stable)
        dummye = consts.tile([1, 1], f32)
        nc.scalar.activation(out=dummye, in_=eps_sq[0:1, :], func=AF.Exp,
                             scale=1.0, bias=nbias[0:1, :])
        nc.vector.reciprocal(out=gsb[:, 1:2], in_=gsd)

        # B2 holds -B = mean*A - beta (sign fixed up at the consumers)
        A2 = consts.tile([128, 2], f32)
        B2 = consts.tile([128, 2], bf16)
        B2f = work.tile([128, 2], f32, name="B2f")
        for i in range(2):
            gbp = pw.tile([128, 2], f32, name="gbp", tag="pw")
            nc.tensor.matmul(gbp, lhsT=gmask8[:, 128 * i:128 * (i + 1)],
                             rhs=gsb, start=True, stop=True)
            nc.vector.tensor_mul(A2[:, i:i + 1], gamma2[:, i:i + 1], gbp[:, 1:2])
            nc.vector.scalar_tensor_tensor(out=B2f[:, i:i + 1], in0=gbp[:, 0:1],
                                           scalar=A2[:, i:i + 1],
                                           in1=beta2[:, i:i + 1], op0=MULT,
                                           op1=mybir.AluOpType.subtract)
        nc.vector.tensor_copy(B2, B2f)
        if DBG:
            dt1 = work.tile([128, 4], f32, name="dt1")
            nc.vector.tensor_copy(dt1, stile)
            nc.sync.dma_start(out=dbg_stile, in_=dt1)
            dt2 = work.tile([128, 4], f32, name="dt2")
            nc.vector.tensor_copy(dt2[:, 0:2], A2)
            nc.vector.tensor_copy(dt2[:, 2:4], B2f)
            nc.sync.dma_start(out=dbg_a2, in_=dt2)

        # ---- fold A into G^T -> fp8 (contraction-side fold for q') ----
        ga8 = consts.tile([128, 2, C], f8)
        for i in range(2):
            nc.vector.tensor_scalar_mul(ga8[:, i, :], gt[:, i, :], A2[:, i:i + 1])

        # ---- delta coefficient: coef = A o (G B + Wk^T bq) ----
        # Folded into the q' drain below: q8 = A*qp + coef makes the single
        # S matmul compute S + delta[m] directly (delta const over n).
        gbv = pw.tile([128, 2], f32, name="gbv", tag="pw")
        for ch in range(2):
            for i in range(2):
                nc.tensor.matmul(gbv[:, ch:ch + 1],
                                 lhsT=gt[:, i, ch * 128:(ch + 1) * 128],
                                 rhs=B2[:, i:i + 1], start=(i == 0), stop=(i == 1))
        coef = work.tile([128, 2], f32, name="coef")
        nc.vector.tensor_sub(coef, wtld2, gbv)     # gbv = -G B
        nc.vector.tensor_mul(coef, coef, A2)

        # ---- q'[c, n] = A[c] * sum_c' (G[c,c'] A[c']) x_q[c', n] + coef[c] ----
        q8 = big.tile([128, 2, NQ], f8)
        for qc in range(2):
            for ch in range(2):
                ns = slice(qc * 512, (qc + 1) * 512)
                qp = psp.tile([128, 512], f32, name="qp", tag="sp")
                nc.tensor.matmul(qp, lhsT=ga8[:, :, ch * 128:(ch + 1) * 128],
                                 rhs=xb8[:, :, ns], start=True, stop=True,
                                 perf_mode=DR)
                nc.vector.tensor_scalar(out=q8[:, ch, ns], in0=qp,
                                        scalar1=A2[:, ch:ch + 1],
                                        scalar2=coef[:, ch:ch + 1],
                                        op0=MULT, op1=ADD)

        if DBG:
            dt3 = work.tile([128, 2, NQ], f32, name="dt3", tag="dbgbig")
            nc.vector.tensor_copy(dt3, q8)
            nc.sync.dma_start(out=dbg_q8, in_=dt3)
            dt4 = work.tile([128, 2, 512], f32, name="dt4", tag="dbgbig2")
            nc.vector.tensor_copy(dt4, coef8b)
            nc.sync.dma_start(out=dbg_coef, in_=dt4)

        # ---- rest of the loads (xb8 keys, weights for biases, xq last) ----
        for i in range(2):
            nc.sync.dma_start(out=xb8[:, i, NQ:N],
                              in_=xb8_d[:, N * i + NQ:N * (i + 1)])
        wall = consts.tile([128, 2, 2 * C], bf16)   # [wv, wp]
        nc.sync.dma_start(out=wall, in_=wall_d)
        pvt = consts.tile([128, 2, C], bf16)
        nc.sync.dma_start(out=pvt, in_=pvt_d)
        xq = big.tile([128, 2, NQ], f32)
        nc.sync.dma_start(out=xq, in_=xq_d)

        # ---- fold A into Pv^T -> fp8 (fused Wp @ Wv projection) ----
        pva8 = consts.tile([128, 2, C], f8)
        for i in range(2):
            nc.vector.tensor_scalar_mul(pva8[:, i, :], pvt[:, i, :],
                                        A2[:, i:i + 1])

        # ---- cv = Wv B + bv (bf16); bpe2 = bp + Wp cv; xqb = xq + bpe2 ----
        cvf = work.tile([128, 2], f32, name="cvf")
        for ot in range(2):
            p = pw.tile([128, 1], f32, name="cvp", tag="pw")
            for i in range(2):
                nc.tensor.matmul(p, lhsT=wall[:, i, 128 * ot:128 * (ot + 1)],
                                 rhs=B2[:, i:i + 1], start=(i == 0), stop=(i == 1))
            nc.vector.tensor_sub(cvf[:, ot:ot + 1], bv2[:, ot:ot + 1], p)
        cv = consts.tile([128, 2], bf16)
        nc.vector.tensor_copy(cv, cvf)
        bpe2 = work.tile([128, 2], f32, name="bpe2")
        for ot in range(2):
            p2 = pw.tile([128, 1], f32, name="bpp", tag="pw")
            for i in range(2):
                nc.tensor.matmul(p2, lhsT=wall[:, i, C + 128 * ot:C + 128 * (ot + 1)],
                                 rhs=cv[:, i:i + 1], start=(i == 0), stop=(i == 1))
            nc.vector.tensor_scalar_add(bpe2[:, ot:ot + 1], p2, bp2[:, ot:ot + 1])
        xqb = big.tile([128, 2, NQ], f32)
        for ot in range(2):
            nc.gpsimd.tensor_scalar_add(xqb[:, ot, :], xq[:, ot, :],
                                        bpe2[:, ot:ot + 1])

        # ---- attention: S + delta -> exp -> r, u0, u1 -> scale -> project ----
        u8 = big.tile([128, 2, NQ], f8)
        for nch in range(2):
            ns = slice(nch * 512, (nch + 1) * 512)
            rp = pr.tile([128, 512], f32, name="rp", tag="r")
            up = [pacc.tile([128, 512], f32, name=f"up{h}", tag="u")
                  for h in range(2)]
            for s in range(16):
                sp = psp.tile([128, 1024], f32, name="sp", tag="sp")
                for h in range(2):
                    ms = slice((2 * s + h) * 128, (2 * s + h + 1) * 128)
                    hs = slice(512 * h, 512 * (h + 1))
                    nc.tensor.matmul(sp[:, hs], lhsT=xb8[:, :, ms],
                                     rhs=q8[:, :, ns], start=True, stop=True,
                                     perf_mode=DR)
                e = epool.tile([128, 2, 512], f8, name="e", tag="e")
                if DBG and nch == 0 and s == 0:
                    dt5 = work.tile([128, 1024], f32, name="dt5", tag="dbgbig3")
                    nc.vector.tensor_copy(dt5, sp)
                    nc.sync.dma_start(out=dbg_sp, in_=dt5)
                nc.scalar.activation(out=e, in_=sp, func=AF.Exp,
                                     scale=1.0 / 16.0, bias=nbias)
                nc.tensor.matmul(rp, lhsT=ones8, rhs=e,
                                 start=(s == 0), stop=(s == 15), perf_mode=DR)
                for h in range(2):
                    nc.tensor.matmul(
                        up[h],
                        lhsT=xt8[:, 2 * s:2 * s + 2, 136 * h:136 * h + 128],
                        rhs=e, start=(s == 0), stop=(s == 15), perf_mode=DR)
            if DBG and nch == 0:
                dt6 = work.tile([128, 512], f32, name="dt6", tag="dbgbig4")
                nc.vector.tensor_copy(dt6, rp)
                nc.sync.dma_start(out=dbg_rp, in_=dt6)
            rb = work.tile([128, 512], f32, name="rb", tag="rb", bufs=2)
            nc.vector.reciprocal(out=rb, in_=rp)
            for h in range(2):
                nc.vector.tensor_mul(u8[:, h, ns], up[h], rb)

            if DBG and nch == 1:
                dt7 = work.tile([128, 2, NQ], f32, name="dt7", tag="dbgbig5")
                nc.vector.tensor_copy(dt7, u8)
                nc.sync.dma_start(out=dbg_u8, in_=dt7)
                dt8 = work.tile([1, C], f32, name="dt8", tag="dbgbig6")
                nc.vector.tensor_copy(dt8, bpeT)
                nc.sync.dma_start(out=dbg_bpe, in_=dt8)
            # fused projection; out = pp/64 + (xq + bpe) in one stt
            for ot in range(2):
                if nch == 1:
                    pp = psp.tile([128, 512], f32, name="pp", tag="sp")
                else:
                    pp = pw.tile([128, 512], f32, name="pp", tag="pw")
                nc.tensor.matmul(pp, lhsT=pva8[:, :, ot * 128:(ot + 1) * 128],
                                 rhs=u8[:, :, ns], start=True, stop=True,
                                 perf_mode=DR)
                ot_t = work.tile([128, 512], f32, name="ot_t", tag="ot_t")
                nc.vector.scalar_tensor_tensor(out=ot_t, in0=pp,
                                               scalar=1.0 / 64.0,
                                               in1=xqb[:, ot, ns], op0=MULT,
                                               op1=ADD)
                nc.sync.dma_start(out=out_d[:, NQ * ot + 512 * nch:
                                            NQ * ot + 512 * (nch + 1)],
                                  in_=ot_t)

    nc.compile()
    return nc


def _get_nc():
    key = "nc"
    if key not in _CACHE:
        _CACHE[key] = _build_nc()
    return _CACHE[key]


def _host_inputs(x, gamma, beta, Wq, bq, Wk, bk, Wv, bv, Wp, bp):
    import ml_dtypes
    f8 = ml_dtypes.float8_e4m3
    bf = ml_dtypes.bfloat16

    x = np.asarray(x, np.float32)
    xf = np.ascontiguousarray(x.reshape(2, C, N))
    gamma = np.asarray(gamma, np.float32)
    beta = np.asarray(beta, np.float32)
    Wq, Wk, Wv, Wp = [np.asarray(W, np.float32) for W in (Wq, Wk, Wv, Wp)]
    bq, bv, bp = [np.asarray(v, np.float32) for v in (bq, bv, bp)]

    # host-fused matrices: G = Wk^T Wq, Pv = Wp @ Wv
    G = Wk.T @ Wq                                             # [c, c']
    Pv = Wp @ Wv                                              # [o, c]
    # gt[p, i, c] = G[c, i*128+p]  (contraction over c' = i*128+p)
    gt = np.ascontiguousarray(
        G.T.reshape(2, 128, C).transpose(1, 0, 2).reshape(128, 2 * C)
    ).astype(bf)
    # pvt[p, i, o] = Pv[o, i*128+p]; the r-matmul's 1/64 stationary makes
    # u8 = 64*u/r (fp8 normal range), repaid by the epilogue's /64.
    pvt = np.ascontiguousarray(
        Pv.T.reshape(2, 128, C).transpose(1, 0, 2).reshape(128, 2 * C)
    ).astype(bf)
    # wall[p, i, (wv|wp), o] = W[o, i*128+p]
    wall = np.stack([Wv.T, Wp.T], axis=1)                     # [c, 2, o]
    wall = wall.reshape(2, 128, 2, C).transpose(1, 0, 2, 3)
    wall = np.ascontiguousarray(wall.reshape(128, 2 * 2 * C)).astype(bf)

    small = np.zeros((128, 26), np.float32)
    small[:, 0:2] = gamma.reshape(2, 128).T
    small[:, 2:4] = beta.reshape(2, 128).T
    small[:, 4:6] = bv.reshape(2, 128).T
    small[:, 6:8] = (Wk.T @ bq).reshape(2, 128).T
    small[:, 8:10] = bp.reshape(2, 128).T
    cids = np.arange(C)
    gm = np.zeros((C, GROUPS), np.float32)
    gm[cids, cids // GSIZE] = 1.0                             # [c, g]
    gmT = gm.reshape(2, 128, GROUPS).transpose(1, 0, 2)       # [p, i, g]
    small[:, 10:18] = gmT[:, 0, :] / (GSIZE * N)
    small[:, 18:26] = gmT[:, 1, :] / (GSIZE * N)
    gmask8 = np.ascontiguousarray(gm.T)                       # [g, c]
    imask = np.eye(128, dtype=np.float32)

    in_maps = []
    for core in range(8):
        b, j = divmod(core, 4)
        xrot = np.roll(xf[b], -j * NQ, axis=1)                # [C, N]
        x8 = xrot.astype(f8)
        xb8 = np.ascontiguousarray(
            x8.reshape(2, 128, N).transpose(1, 0, 2).reshape(128, 2 * N))
        xqv = np.ascontiguousarray(
            xrot[:, :NQ].reshape(2, 128, NQ).transpose(1, 0, 2)
            .reshape(128, 2 * NQ))
        # xt8: position-major with ones cols: per n: [c0..c127, 1, c128.., 1]
        xt = x8.astype(np.float32).T                          # [n, c] quantized
        arr = np.zeros((N, 272), np.float32)
        arr[:, 0:128] = xt[:, 0:128]
        arr[:, 128] = 1.0
        arr[:, 136:264] = xt[:, 128:256]
        arr[:, 264] = 1.0
        xt8 = np.ascontiguousarray(
            arr.reshape(32, 128, 272).transpose(1, 0, 2).reshape(128, 32 * 272)
        ).astype(f8)
        in_maps.append({
            "xt8": xt8, "xb8": xb8, "gt": gt, "pvt": pvt, "wall": wall,
            "small": small, "gmask8": gmask8, "imask": imask,
            "xq": xqv,
        })
    return in_maps


def kernel(x, gamma, beta, Wq, bq, Wk, bk, Wv, bv, Wp, bp):
    from concourse.bass_utils import run_bass_kernel_spmd
    global LAST_RESULTS

    orig_shape = np.asarray(x).shape
    in_maps = _host_inputs(x, gamma, beta, Wq, bq, Wk, bk, Wv, bv, Wp, bp)
    nc = _get_nc()

    trace = os.environ.get("BASSK_TRACE", "0") == "1"
    res = run_bass_kernel_spmd(nc, in_maps, core_ids=list(range(8)), trace=trace)
    LAST_RESULTS = res

    out = np.empty((2, C, N), np.float32)
    for core in range(8):
        b, j = divmod(core, 4)
        o = res.results[core]["out"]                          # [128, 2*NQ]
        o = o.reshape(128, 2, NQ).transpose(1, 0, 2).reshape(C, NQ)
        out[b][:, j * NQ:(j + 1) * NQ] = o
    return out.reshape(orig_shape)


# revision 28
# speedup vs baseline: 2.1064x; 1.0162x over previous
"""AttentionBlock3D (GroupNorm + single-head self-attention + residual) on 8 TRN2 cores.

Sharding: core = (batch b in {0,1}) x (1024-row slice of the 4096 attention rows).
Each core computes its batch's GroupNorm stats (cheap, on the PE) and
attention + output projection for its own 1024 query rows. No collectives.
The host ROTATES each core's x copy so that its query rows are always
columns 0..1024 (attention is permutation-invariant over keys).

fp8 DoubleRow pipeline with K and V eliminated:
  - x ships twice in fp8 e4m3: xb8 [128, 2, N] (channel-major) and
    xt8 [128, 32, 258] (position-major with built-in ones columns).
  - GroupNorm stats on the PE: x^T x accumulated per channel half; the ones
    column gives Sum x, the diagonal (mask reduce) gives Sum x^2.
  - S = hn^T G hn_q with G = Wk^T Wq (host): q' = (G.A-folded)^T x_q scaled
    by A, S = x^T q' via DoubleRow with xb8 stationary — no K tensor.
    Per-key bias delta[m] = (A o (G B + Wk^T bq))^T x[:,m] enters S as one
    extra DoubleRow matmul with constant-column rhs (coef broadcast).
  - u = x^T E via DoubleRow with xt8 stationary — no V tensor; the output
    projection fuses Pv = Wp @ Wv (host) with A folded on device:
    out = x_q + bpe + (Pv.A)^T u / r,  bpe = bp + Wp cv, cv = Wv B + bv.
  - E = exp(S/16 - 2) on ACT in [128,1024] tiles (e^-2 cancels against r).
    ACT exp is the bottleneck engine (~4.2M elements/core).
"""

import os
import numpy as np
from contextlib import ExitStack

C = 256          # channels
N = 4096         # spatial positions (16*16*16)
NQ = 1024        # query rows per core
GROUPS = 8
GSIZE = C // GROUPS
EPS = 1e-5

_CACHE = {}
LAST_RESULTS = None  # test harness can inspect trace results


def _build_nc():
    import concourse.bacc as bacc
    import concourse.tile as tile
    from concourse import mybir

    f32 = mybir.dt.float32
    f32r = mybir.dt.float32r
    bf16 = mybir.dt.bfloat16
    f8 = mybir.dt.float8e4
    AF = mybir.ActivationFunctionType
    DR = mybir.MatmulPerfMode.DoubleRow
    ADD = mybir.AluOpType.add
    MULT = mybir.AluOpType.mult

    nc = bacc.Bacc("TRN2", target_bir_lowering=False, debug=False,
                   enable_asserts=False)

    # ---- DRAM I/O (per-core) ----
    xt8_d = nc.dram_tensor("xt8", [128, 32 * 272], f8, kind="ExternalInput").ap()
    xb8_d = nc.dram_tensor("xb8", [128, 2 * N], f8, kind="ExternalInput").ap()
    gt_d = nc.dram_tensor("gt", [128, 2 * C], bf16, kind="ExternalInput").ap()
    pvt_d = nc.dram_tensor("pvt", [128, 2 * C], bf16, kind="ExternalInput").ap()
    wall_d = nc.dram_tensor("wall", [128, 2 * 2 * C], bf16, kind="ExternalInput").ap()
    small_d = nc.dram_tensor("small", [128, 26], f32, kind="ExternalInput").ap()
    gmask8_d = nc.dram_tensor("gmask8", [GROUPS, C], f32, kind="ExternalInput").ap()
    imask_d = nc.dram_tensor("imask", [128, 128], bf16, kind="ExternalInput").ap()
    dxq_d = nc.dram_tensor("dxq", [128, 2 * NQ], bf16, kind="ExternalInput").ap()
    out_d = nc.dram_tensor("out", [128, 2 * NQ], f32, kind="ExternalOutput").ap()
    DBG = os.environ.get("BASSK_DBG", "0") == "1"
    if DBG:
        dbg_stile = nc.dram_tensor("dbg_stile", [128, 4], f32, kind="ExternalOutput").ap()
        dbg_a2 = nc.dram_tensor("dbg_a2", [128, 4], f32, kind="ExternalOutput").ap()
        dbg_q8 = nc.dram_tensor("dbg_q8", [128, 2 * NQ], f32, kind="ExternalOutput").ap()
        dbg_coef = nc.dram_tensor("dbg_coef", [128, 2 * 512], f32, kind="ExternalOutput").ap()
        dbg_sp = nc.dram_tensor("dbg_sp", [128, 1024], f32, kind="ExternalOutput").ap()
        dbg_rp = nc.dram_tensor("dbg_rp", [128, 512], f32, kind="ExternalOutput").ap()
        dbg_u8 = nc.dram_tensor("dbg_u8", [128, 2 * NQ], f32, kind="ExternalOutput").ap()
        dbg_bpe = nc.dram_tensor("dbg_bpe", [1, C], f32, kind="ExternalOutput").ap()

    with tile.TileContext(nc) as tc, ExitStack() as ctx:
        big = ctx.enter_context(tc.tile_pool(name="big", bufs=1))
        consts = ctx.enter_context(tc.tile_pool(name="consts", bufs=1))
        work = ctx.enter_context(tc.tile_pool(name="work", bufs=3))
        epool = ctx.enter_context(tc.tile_pool(name="epool", bufs=3))
        # PSUM banks: psp 2x2 + pacc 2 + pr 1 + pw 1 = 8
        psp = ctx.enter_context(tc.tile_pool(name="psp", bufs=2, space="PSUM"))
        pacc = ctx.enter_context(tc.tile_pool(name="pacc", bufs=2, space="PSUM"))
        pr = ctx.enter_context(tc.tile_pool(name="pr", bufs=1, space="PSUM"))
        pw = ctx.enter_context(tc.tile_pool(name="pw", bufs=1, space="PSUM"))

        # ---- constants (before the big loads) ----
        # r-matmul stationary = 1/64 so the reciprocal yields 64/r; the /64
        # is repaid in the epilogue stt. Keeps u8 = 64*u/r in fp8's sweet spot.
        ones8 = consts.tile([128, 2, 128], f8)
        nc.vector.memset(ones8, 1.0 / 64.0)
        nbias = consts.tile([128, 1], f32)
        nc.vector.memset(nbias, -3.5)
        # eps8 = Sqrt(EPS^2) on ACT: forces the Sqrt act-table load at t~0
        eps_sq = consts.tile([GROUPS, 1], f32)
        nc.vector.memset(eps_sq, EPS * EPS)
        eps8 = consts.tile([GROUPS, 1], f32)
        nc.scalar.activation(out=eps8, in_=eps_sq, func=AF.Sqrt, scale=1.0)

        # ---- load xt8 (stats + u path) chunked; stats matmuls interleave ----
        xt8 = big.tile([128, 32, 272], f8)
        for ch in range(4):
            nc.sync.dma_start(out=xt8[:, 8 * ch:8 * ch + 8, :],
                              in_=xt8_d[:, 8 * 272 * ch:8 * 272 * (ch + 1)])
        # xx[h] accumulates x^T x for channel half h ([128,129]: 128 cols of
        # the x^T x block + ones column giving Sum x). Uses the idle sp slots.
        xx = []
        for h in range(2):
            t = psp.tile([128, 129], f32, name=f"xx{h}", tag="sp")
            xx.append(t)
        for s in range(16):
            for h in range(2):
                nc.tensor.matmul(
                    xx[h],
                    lhsT=xt8[:, 2 * s:2 * s + 2, 136 * h:136 * h + 128],
                    rhs=xt8[:, 2 * s:2 * s + 2, 136 * h:136 * h + 129],
                    start=(s == 0), stop=(s == 15), perf_mode=DR)

        # ---- smalls + G^T + xb8 query cols (early, for q') ----
        small_sb = consts.tile([128, 26], f32)
        nc.sync.dma_start(out=small_sb, in_=small_d)
        gamma2 = small_sb[:, 0:2]
        beta2 = small_sb[:, 2:4]
        bv2 = small_sb[:, 4:6]
        wtld2 = small_sb[:, 6:8]          # Wk^T bq (host)
        bp2 = small_sb[:, 8:10]
        gmaskT = [small_sb[:, 10 + 8 * i:18 + 8 * i] for i in range(2)]  # pre-scaled 1/(32N)
        imask = consts.tile([128, 128], bf16)
        nc.sync.dma_start(out=imask, in_=imask_d)
        gmask8 = consts.tile([GROUPS, C], f32)
        nc.sync.dma_start(out=gmask8, in_=gmask8_d)
        gt = consts.tile([128, 2, C], bf16)
        nc.sync.dma_start(out=gt, in_=gt_d)
        xb8 = big.tile([128, 2, N], f8)
        for i in range(2):
            nc.sync.dma_start(out=xb8[:, i, 0:NQ], in_=xb8_d[:, N * i:N * i + NQ])

        # ---- group stats -> per-channel A (f32) and B (bf16) ----
        # Sum x^2 = diag(x^T x): mask out the diagonal, then column-sum it
        # back to [128,1] with a N=1 matmul (diag matrix -> col sums = diag).
        ones_col = consts.tile([128, 2], f32)
        nc.vector.memset(ones_col, 1.0)
        ones_colr = consts.tile([128, 2], f32r)
        nc.vector.tensor_copy(ones_colr, ones_col)
        stile = work.tile([128, 2, 2], f32, name="stile")  # [:, h, (sx, sxx)]
        for h in range(2):
            nc.vector.tensor_copy(stile[:, h, 0:1], xx[h][:, 128:129])
            scr = work.tile([128, 128], f32r, name="scr", tag="scr", bufs=2)
            nc.vector.tensor_mul(scr, xx[h][:, 0:128], imask)
            sxpool = pw if h == 0 else pr
            sxp = sxpool.tile([128, 2], f32, name="sxp", tag="pw" if h == 0 else "r")
            nc.tensor.matmul(sxp, lhsT=scr, rhs=ones_colr, start=True, stop=True)
            nc.vector.tensor_copy(stile[:, h, 1:2], sxp[:, 0:1])
        gp = pacc.tile([GROUPS, 2], f32, tag="u")
        for h in range(2):
            nc.tensor.matmul(gp, lhsT=gmaskT[h], rhs=stile[:, h, :],
                             start=(h == 0), stop=(h == 1))
        # gmaskT is host-scaled by 1/(32N), so gp = (mean, E[x^2]) directly.
        gsb = work.tile([GROUPS, 2], f32, name="gsb")
        nc.vector.tensor_copy(gsb, gp)
        negvar = work.tile([GROUPS, 1], f32, name="negvar")
        nc.vector.scalar_tensor_tensor(out=negvar, in0=gsb[:, 0:1],
                                       scalar=gsb[:, 0:1], in1=gsb[:, 1:2],
                                       op0=MULT, op1=mybir.AluOpType.subtract)
        gsd = work.tile([GROUPS, 1], f32, name="gsd")
        nc.scalar.activation(out=gsd, in_=negvar, func=AF.Sqrt, bias=eps8,
                             scale=-1.0)
        # preload the Exp act table now (the Sqrt above was ACT's last
        # non-Exp op; loading here keeps the main loop table-stable)
        dummye = consts.tile([1, 1], f32)
        nc.scalar.activation(out=dummye, in_=gsd[0:1, :], func=AF.Exp,
                             scale=1.0, bias=nbias[0:1, :])
        nc.vector.reciprocal(out=gsb[:, 1:2], in_=gsd)

        # B2 holds -B = mean*A - beta (sign fixed up at the consumers)
        A2 = consts.tile([128, 2], f32)
        B2 = consts.tile([128, 2], bf16)
        B2f = work.tile([128, 2], f32, name="B2f")
        for i in range(2):
            gbp = pw.tile([128, 2], f32, name="gbp", tag="pw")
            nc.tensor.matmul(gbp, lhsT=gmask8[:, 128 * i:128 * (i + 1)],
                             rhs=gsb, start=True, stop=True)
            nc.vector.tensor_mul(A2[:, i:i + 1], gamma2[:, i:i + 1], gbp[:, 1:2])
            nc.vector.scalar_tensor_tensor(out=B2f[:, i:i + 1], in0=gbp[:, 0:1],
                                           scalar=A2[:, i:i + 1],
                                           in1=beta2[:, i:i + 1], op0=MULT,
                                           op1=mybir.AluOpType.subtract)
        nc.vector.tensor_copy(B2, B2f)
        if DBG:
            dt1 = work.tile([128, 4], f32, name="dt1")
            nc.vector.tensor_copy(dt1, stile)
            nc.sync.dma_start(out=dbg_stile, in_=dt1)
            dt2 = work.tile([128, 4], f32, name="dt2")
            nc.vector.tensor_copy(dt2[:, 0:2], A2)
            nc.vector.tensor_copy(dt2[:, 2:4], B2f)
            nc.sync.dma_start(out=dbg_a2, in_=dt2)

        # ---- fold A into G^T -> fp8 (contraction-side fold for q') ----
        ga8 = consts.tile([128, 2, C], f8)
        for i in range(2):
            nc.vector.tensor_scalar_mul(ga8[:, i, :], gt[:, i, :], A2[:, i:i + 1])

        # ---- delta coefficient: coef = A o (G B + Wk^T bq) ----
        # Folded into the q' drain below: q8 = A*qp + coef makes the single
        # S matmul compute S + delta[m] directly (delta const over n).
        gbv = pacc.tile([128, 2], f32, name="gbv", tag="u")
        for ch in range(2):
            for i in range(2):
                nc.tensor.matmul(gbv[:, ch:ch + 1],
                                 lhsT=gt[:, i, ch * 128:(ch + 1) * 128],
                                 rhs=B2[:, i:i + 1], start=(i == 0), stop=(i == 1))
        coef = work.tile([128, 2], f32, name="coef")
        nc.vector.tensor_sub(coef, wtld2, gbv)     # gbv = -G B
        nc.vector.tensor_mul(coef, coef, A2)

        # ---- q'[c, n] = A[c] * sum_c' (G[c,c'] A[c']) x_q[c', n] + coef[c] ----
        q8 = big.tile([128, 2, NQ], f8)

        def emit_q(qc):
            for ch in range(2):
                ns = slice(qc * 512, (qc + 1) * 512)
                qp = psp.tile([128, 512], f32, name="qp", tag="sp")
                nc.tensor.matmul(qp, lhsT=ga8[:, :, ch * 128:(ch + 1) * 128],
                                 rhs=xb8[:, :, ns], start=True, stop=True,
                                 perf_mode=DR)
                nc.vector.tensor_scalar(out=q8[:, ch, ns], in0=qp,
                                        scalar1=A2[:, ch:ch + 1],
                                        scalar2=coef[:, ch:ch + 1],
                                        op0=MULT, op1=ADD)

        emit_q(0)

        if DBG:
            dt3 = work.tile([128, 2, NQ], f32, name="dt3", tag="dbgbig")
            nc.vector.tensor_copy(dt3, q8)
            nc.sync.dma_start(out=dbg_q8, in_=dt3)
            dt4 = work.tile([128, 2, 512], f32, name="dt4", tag="dbgbig2")
            nc.vector.tensor_copy(dt4, coef8b)
            nc.sync.dma_start(out=dbg_coef, in_=dt4)

        # ---- rest of the loads (xb8 keys, weights for biases, xq last) ----
        for i in range(2):
            nc.sync.dma_start(out=xb8[:, i, NQ:N],
                              in_=xb8_d[:, N * i + NQ:N * (i + 1)])
        dxq = big.tile([128, 2, NQ], bf16)
        nc.sync.dma_start(out=dxq, in_=dxq_d)
        wall = consts.tile([128, 2, 2 * C], bf16)   # [wv, wp]
        nc.sync.dma_start(out=wall, in_=wall_d)
        pvt = consts.tile([128, 2, C], bf16)
        nc.sync.dma_start(out=pvt, in_=pvt_d)

        # ---- fold A into Pv^T -> fp8 (fused Wp @ Wv projection) ----
        pva8 = consts.tile([128, 2, C], f8)
        for i in range(2):
            nc.vector.tensor_scalar_mul(pva8[:, i, :], pvt[:, i, :],
                                        A2[:, i:i + 1])

        # ---- cv = Wv B + bv (bf16); bpe2 = bp + Wp cv; xqb = xq + bpe2 ----
        cvf = work.tile([128, 2], f32, name="cvf")
        for ot in range(2):
            cvpool = pw if ot == 0 else pr
            p = cvpool.tile([128, 1], f32, name="cvp",
                            tag="pw" if ot == 0 else "r")
            for i in range(2):
                nc.tensor.matmul(p, lhsT=wall[:, i, 128 * ot:128 * (ot + 1)],
                                 rhs=B2[:, i:i + 1], start=(i == 0), stop=(i == 1))
            nc.vector.tensor_sub(cvf[:, ot:ot + 1], bv2[:, ot:ot + 1], p)
        cv = consts.tile([128, 2], bf16)
        nc.vector.tensor_copy(cv, cvf)
        bpe2 = work.tile([128, 2], f32, name="bpe2")
        for ot in range(2):
            bppool = pw if ot == 0 else pr
            p2 = bppool.tile([128, 1], f32, name="bpp",
                             tag="pw" if ot == 0 else "r")
            for i in range(2):
                nc.tensor.matmul(p2, lhsT=wall[:, i, C + 128 * ot:C + 128 * (ot + 1)],
                                 rhs=cv[:, i:i + 1], start=(i == 0), stop=(i == 1))
            nc.vector.tensor_scalar_add(bpe2[:, ot:ot + 1], p2, bp2[:, ot:ot + 1])
        # residual x_q reconstructed as fp8(x) + bf16 quantization remainder
        xqb = big.tile([128, 2, NQ], f32)
        for ot in range(2):
            nc.gpsimd.tensor_add(xqb[:, ot, :], xb8[:, ot, 0:NQ], dxq[:, ot, :])
            nc.gpsimd.tensor_scalar_add(xqb[:, ot, :], xqb[:, ot, :],
                                        bpe2[:, ot:ot + 1])

        # ---- attention: S + delta -> exp -> r, u0, u1 -> scale -> project ----
        u8 = big.tile([128, 2, NQ], f8)
        for nch in range(2):
            ns = slice(nch * 512, (nch + 1) * 512)
            rp = pr.tile([128, 512], f32, name="rp", tag="r")
            up = [pacc.tile([128, 512], f32, name=f"up{h}", tag="u")
                  for h in range(2)]
            for s in range(16):
                if nch == 0 and s == 2:
                    emit_q(1)
                sp = psp.tile([128, 1024], f32, name="sp", tag="sp")
                for h in range(2):
                    ms = slice((2 * s + h) * 128, (2 * s + h + 1) * 128)
                    hs = slice(512 * h, 512 * (h + 1))
                    nc.tensor.matmul(sp[:, hs], lhsT=xb8[:, :, ms],
                                     rhs=q8[:, :, ns], start=True, stop=True,
                                     perf_mode=DR)
                e = epool.tile([128, 2, 512], f8, name="e", tag="e")
                if DBG and nch == 0 and s == 0:
                    dt5 = work.tile([128, 1024], f32, name="dt5", tag="dbgbig3")
                    nc.vector.tensor_copy(dt5, sp)
                    nc.sync.dma_start(out=dbg_sp, in_=dt5)
                nc.scalar.activation(out=e, in_=sp, func=AF.Exp,
                                     scale=1.0 / 16.0, bias=nbias)
                nc.tensor.matmul(rp, lhsT=ones8, rhs=e,
                                 start=(s == 0), stop=(s == 15), perf_mode=DR)
                for h in range(2):
                    nc.tensor.matmul(
                        up[h],
                        lhsT=xt8[:, 2 * s:2 * s + 2, 136 * h:136 * h + 128],
                        rhs=e, start=(s == 0), stop=(s == 15), perf_mode=DR)
            if DBG and nch == 0:
                dt6 = work.tile([128, 512], f32, name="dt6", tag="dbgbig4")
                nc.vector.tensor_copy(dt6, rp)
                nc.sync.dma_start(out=dbg_rp, in_=dt6)
            rb = work.tile([128, 512], f32, name="rb", tag="rb", bufs=2)
            nc.vector.reciprocal(out=rb, in_=rp)
            for h in range(2):
                nc.vector.tensor_mul(u8[:, h, ns], up[h], rb)

            if DBG and nch == 1:
                dt7 = work.tile([128, 2, NQ], f32, name="dt7", tag="dbgbig5")
                nc.vector.tensor_copy(dt7, u8)
                nc.sync.dma_start(out=dbg_u8, in_=dt7)
                dt8 = work.tile([1, C], f32, name="dt8", tag="dbgbig6")
                nc.vector.tensor_copy(dt8, bpeT)
                nc.sync.dma_start(out=dbg_bpe, in_=dt8)
            # fused projection; out = pp/64 + (xq + bpe) in one stt
            for ot in range(2):
                if nch == 1:
                    pp = psp.tile([128, 512], f32, name="pp", tag="sp")
                else:
                    pp = pw.tile([128, 512], f32, name="pp", tag="pw")
                nc.tensor.matmul(pp, lhsT=pva8[:, :, ot * 128:(ot + 1) * 128],
                                 rhs=u8[:, :, ns], start=True, stop=True,
                                 perf_mode=DR)
                ot_t = work.tile([128, 512], f32, name="ot_t", tag="ot_t")
                nc.vector.scalar_tensor_tensor(out=ot_t, in0=pp,
                                               scalar=1.0 / 64.0,
                                               in1=xqb[:, ot, ns], op0=MULT,
                                               op1=ADD)
                nc.sync.dma_start(out=out_d[:, NQ * ot + 512 * nch:
                                            NQ * ot + 512 * (nch + 1)],
                                  in_=ot_t)

    nc.compile()
    return nc


def _get_nc():
    key = "nc"
    if key not in _CACHE:
        _CACHE[key] = _build_nc()
    return _CACHE[key]


def _host_inputs(x, gamma, beta, Wq, bq, Wk, bk, Wv, bv, Wp, bp):
    import ml_dtypes
    f8 = ml_dtypes.float8_e4m3
    bf = ml_dtypes.bfloat16

    x = np.asarray(x, np.float32)
    xf = np.ascontiguousarray(x.reshape(2, C, N))
    gamma = np.asarray(gamma, np.float32)
    beta = np.asarray(beta, np.float32)
    Wq, Wk, Wv, Wp = [np.asarray(W, np.float32) for W in (Wq, Wk, Wv, Wp)]
    bq, bv, bp = [np.asarray(v, np.float32) for v in (bq, bv, bp)]

    # host-fused matrices: G = Wk^T Wq, Pv = Wp @ Wv
    G = Wk.T @ Wq                                             # [c, c']
    Pv = Wp @ Wv                                              # [o, c]
    # gt[p, i, c] = G[c, i*128+p]  (contraction over c' = i*128+p)
    gt = np.ascontiguousarray(
        G.T.reshape(2, 128, C).transpose(1, 0, 2).reshape(128, 2 * C)
    ).astype(bf)
    # pvt[p, i, o] = Pv[o, i*128+p]; the r-matmul's 1/64 stationary makes
    # u8 = 64*u/r (fp8 normal range), repaid by the epilogue's /64.
    pvt = np.ascontiguousarray(
        Pv.T.reshape(2, 128, C).transpose(1, 0, 2).reshape(128, 2 * C)
    ).astype(bf)
    # wall[p, i, (wv|wp), o] = W[o, i*128+p]
    wall = np.stack([Wv.T, Wp.T], axis=1)                     # [c, 2, o]
    wall = wall.reshape(2, 128, 2, C).transpose(1, 0, 2, 3)
    wall = np.ascontiguousarray(wall.reshape(128, 2 * 2 * C)).astype(bf)

    small = np.zeros((128, 26), np.float32)
    small[:, 0:2] = gamma.reshape(2, 128).T
    small[:, 2:4] = beta.reshape(2, 128).T
    small[:, 4:6] = bv.reshape(2, 128).T
    small[:, 6:8] = (Wk.T @ bq).reshape(2, 128).T
    small[:, 8:10] = bp.reshape(2, 128).T
    cids = np.arange(C)
    gm = np.zeros((C, GROUPS), np.float32)
    gm[cids, cids // GSIZE] = 1.0                             # [c, g]
    gmT = gm.reshape(2, 128, GROUPS).transpose(1, 0, 2)       # [p, i, g]
    small[:, 10:18] = gmT[:, 0, :] / (GSIZE * N)
    small[:, 18:26] = gmT[:, 1, :] / (GSIZE * N)
    gmask8 = np.ascontiguousarray(gm.T)                       # [g, c]
    imask = np.eye(128, dtype=np.float32).astype(bf)

    in_maps = []
    for core in range(8):
        b, j = divmod(core, 4)
        xrot = np.roll(xf[b], -j * NQ, axis=1)                # [C, N]
        x8 = xrot.astype(f8)
        xb8 = np.ascontiguousarray(
            x8.reshape(2, 128, N).transpose(1, 0, 2).reshape(128, 2 * N))
        dxq = (xrot[:, :NQ] - x8[:, :NQ].astype(np.float32))
        dxq = np.ascontiguousarray(
            dxq.reshape(2, 128, NQ).transpose(1, 0, 2).reshape(128, 2 * NQ)
        ).astype(bf)
        # xt8: position-major with ones cols: per n: [c0..c127, 1, c128.., 1]
        xt = x8.astype(np.float32).T                          # [n, c] quantized
        arr = np.zeros((N, 272), np.float32)
        arr[:, 0:128] = xt[:, 0:128]
        arr[:, 128] = 1.0
        arr[:, 136:264] = xt[:, 128:256]
        arr[:, 264] = 1.0
        xt8 = np.ascontiguousarray(
            arr.reshape(32, 128, 272).transpose(1, 0, 2).reshape(128, 32 * 272)
        ).astype(f8)
        in_maps.append({
            "xt8": xt8, "xb8": xb8, "gt": gt, "pvt": pvt, "wall": wall,
            "small": small, "gmask8": gmask8, "imask": imask,
            "dxq": dxq,
        })
    return in_maps


def kernel(x, gamma, beta, Wq, bq, Wk, bk, Wv, bv, Wp, bp):
    from concourse.bass_utils import run_bass_kernel_spmd
    global LAST_RESULTS

    orig_shape = np.asarray(x).shape
    in_maps = _host_inputs(x, gamma, beta, Wq, bq, Wk, bk, Wv, bv, Wp, bp)
    nc = _get_nc()

    trace = os.environ.get("BASSK_TRACE", "0") == "1"
    res = run_bass_kernel_spmd(nc, in_maps, core_ids=list(range(8)), trace=trace)
    LAST_RESULTS = res

    out = np.empty((2, C, N), np.float32)
    for core in range(8):
        b, j = divmod(core, 4)
        o = res.results[core]["out"]                          # [128, 2*NQ]
        o = o.reshape(128, 2, NQ).transpose(1, 0, 2).reshape(C, NQ)
        out[b][:, j * NQ:(j + 1) * NQ] = o
    return out.reshape(orig_shape)


# revision 32
# speedup vs baseline: 2.1413x; 1.0166x over previous
"""AttentionBlock3D (GroupNorm + single-head self-attention + residual) on 8 TRN2 cores.

Sharding: core = (batch b in {0,1}) x (1024-row slice of the 4096 attention rows).
Each core computes its batch's GroupNorm stats (cheap, on the PE) and
attention + output projection for its own 1024 query rows. No collectives.
The host ROTATES each core's x copy so that its query rows are always
columns 0..1024 (attention is permutation-invariant over keys).

fp8 DoubleRow pipeline with K and V eliminated:
  - x ships twice in fp8 e4m3: xb8 [128, 2, N] (channel-major) and
    xt8 [128, 32, 258] (position-major with built-in ones columns).
  - GroupNorm stats on the PE: x^T x accumulated per channel half; the ones
    column gives Sum x, the diagonal (mask reduce) gives Sum x^2.
  - S = hn^T G hn_q with G = Wk^T Wq (host): q' = (G.A-folded)^T x_q scaled
    by A, S = x^T q' via DoubleRow with xb8 stationary — no K tensor.
    Per-key bias delta[m] = (A o (G B + Wk^T bq))^T x[:,m] enters S as one
    extra DoubleRow matmul with constant-column rhs (coef broadcast).
  - u = x^T E via DoubleRow with xt8 stationary — no V tensor; the output
    projection fuses Pv = Wp @ Wv (host) with A folded on device:
    out = x_q + bpe + (Pv.A)^T u / r,  bpe = bp + Wp cv, cv = Wv B + bv.
  - E = exp(S/16 - 2) on ACT in [128,1024] tiles (e^-2 cancels against r).
    ACT exp is the bottleneck engine (~4.2M elements/core).
"""

import os
import numpy as np
from contextlib import ExitStack

C = 256          # channels
N = 4096         # spatial positions (16*16*16)
NQ = 1024        # query rows per core
GROUPS = 8
GSIZE = C // GROUPS
EPS = 1e-5

_CACHE = {}
LAST_RESULTS = None  # test harness can inspect trace results


def _build_nc():
    import concourse.bacc as bacc
    import concourse.tile as tile
    from concourse import mybir

    f32 = mybir.dt.float32
    f32r = mybir.dt.float32r
    bf16 = mybir.dt.bfloat16
    f8 = mybir.dt.float8e4
    AF = mybir.ActivationFunctionType
    DR = mybir.MatmulPerfMode.DoubleRow
    ADD = mybir.AluOpType.add
    MULT = mybir.AluOpType.mult

    nc = bacc.Bacc("TRN2", target_bir_lowering=False, debug=False,
                   enable_asserts=False)

    # ---- DRAM I/O (per-core) ----
    xt8_d = nc.dram_tensor("xt8", [128, 32 * 272], f8, kind="ExternalInput").ap()
    xb8_d = nc.dram_tensor("xb8", [128, 2 * N], f8, kind="ExternalInput").ap()
    gt_d = nc.dram_tensor("gt", [128, 2 * C], bf16, kind="ExternalInput").ap()
    pvt_d = nc.dram_tensor("pvt", [128, 2 * C], bf16, kind="ExternalInput").ap()
    wall_d = nc.dram_tensor("wall", [128, 2 * 2 * C], bf16, kind="ExternalInput").ap()
    small_d = nc.dram_tensor("small", [128, 26], f32, kind="ExternalInput").ap()
    gmask8_d = nc.dram_tensor("gmask8", [GROUPS, C], f32, kind="ExternalInput").ap()
    imask_d = nc.dram_tensor("imask", [128, 128], bf16, kind="ExternalInput").ap()
    dxq_d = nc.dram_tensor("dxq", [128, 2 * NQ], bf16, kind="ExternalInput").ap()
    out_d = nc.dram_tensor("out", [128, 2 * NQ], f32, kind="ExternalOutput").ap()
    DBG = os.environ.get("BASSK_DBG", "0") == "1"
    if DBG:
        dbg_stile = nc.dram_tensor("dbg_stile", [128, 4], f32, kind="ExternalOutput").ap()
        dbg_a2 = nc.dram_tensor("dbg_a2", [128, 4], f32, kind="ExternalOutput").ap()
        dbg_q8 = nc.dram_tensor("dbg_q8", [128, 2 * NQ], f32, kind="ExternalOutput").ap()
        dbg_coef = nc.dram_tensor("dbg_coef", [128, 2 * 512], f32, kind="ExternalOutput").ap()
        dbg_sp = nc.dram_tensor("dbg_sp", [128, 1024], f32, kind="ExternalOutput").ap()
        dbg_rp = nc.dram_tensor("dbg_rp", [128, 512], f32, kind="ExternalOutput").ap()
        dbg_u8 = nc.dram_tensor("dbg_u8", [128, 2 * NQ], f32, kind="ExternalOutput").ap()
        dbg_bpe = nc.dram_tensor("dbg_bpe", [1, C], f32, kind="ExternalOutput").ap()

    with tile.TileContext(nc) as tc, ExitStack() as ctx:
        big = ctx.enter_context(tc.tile_pool(name="big", bufs=1))
        consts = ctx.enter_context(tc.tile_pool(name="consts", bufs=1))
        work = ctx.enter_context(tc.tile_pool(name="work", bufs=3))
        epool = ctx.enter_context(tc.tile_pool(name="epool", bufs=3))
        # PSUM banks: psp 2x2 + pacc 2 + pr 1 + pw 1 = 8
        psp = ctx.enter_context(tc.tile_pool(name="psp", bufs=2, space="PSUM"))
        pacc = ctx.enter_context(tc.tile_pool(name="pacc", bufs=2, space="PSUM"))
        pr = ctx.enter_context(tc.tile_pool(name="pr", bufs=1, space="PSUM"))
        pw = ctx.enter_context(tc.tile_pool(name="pw", bufs=1, space="PSUM"))

        # ---- constants (before the big loads) ----
        # r-matmul stationary = 1/64 so the reciprocal yields 64/r; the /64
        # is repaid in the epilogue stt. Keeps u8 = 64*u/r in fp8's sweet spot.
        ones8 = consts.tile([128, 2, 128], f8)
        nc.vector.memset(ones8, 1.0 / 64.0)
        nbias = consts.tile([128, 1], f32)
        nc.vector.memset(nbias, -3.5)
        # eps8 = Sqrt(EPS^2) on ACT: forces the Sqrt act-table load at t~0
        eps_sq = consts.tile([GROUPS, 1], f32)
        nc.vector.memset(eps_sq, EPS * EPS)
        eps8 = consts.tile([GROUPS, 1], f32)
        nc.scalar.activation(out=eps8, in_=eps_sq, func=AF.Sqrt, scale=1.0)

        # ---- load xt8 (stats + u path) chunked; stats matmuls interleave ----
        xt8 = big.tile([128, 32, 272], f8)
        for ch in range(4):
            nc.sync.dma_start(out=xt8[:, 8 * ch:8 * ch + 8, :],
                              in_=xt8_d[:, 8 * 272 * ch:8 * 272 * (ch + 1)])
        # xx[h] accumulates x^T x for channel half h ([128,129]: 128 cols of
        # the x^T x block + ones column giving Sum x). Uses the idle sp slots.
        xx = []
        for h in range(2):
            t = psp.tile([128, 129], f32, name=f"xx{h}", tag="sp")
            xx.append(t)
        for s in range(16):
            for h in range(2):
                nc.tensor.matmul(
                    xx[h],
                    lhsT=xt8[:, 2 * s:2 * s + 2, 136 * h:136 * h + 128],
                    rhs=xt8[:, 2 * s:2 * s + 2, 136 * h:136 * h + 129],
                    start=(s == 0), stop=(s == 15), perf_mode=DR)

        # ---- smalls + G^T + xb8 query cols (early, for q') ----
        small_sb = consts.tile([128, 26], f32)
        nc.sync.dma_start(out=small_sb, in_=small_d)
        imask = consts.tile([128, 128], bf16)
        nc.sync.dma_start(out=imask, in_=imask_d)
        gmask8 = consts.tile([GROUPS, C], f32)
        nc.sync.dma_start(out=gmask8, in_=gmask8_d)
        gamma2 = small_sb[:, 0:2]
        beta2 = small_sb[:, 2:4]
        bv2 = small_sb[:, 4:6]
        wtld2 = small_sb[:, 6:8]          # Wk^T bq (host)
        bp2 = small_sb[:, 8:10]
        gmaskT = [small_sb[:, 10 + 8 * i:18 + 8 * i] for i in range(2)]  # pre-scaled 1/(32N)
        gt = consts.tile([128, 2, C], bf16)
        nc.sync.dma_start(out=gt, in_=gt_d)
        xb8 = big.tile([128, 2, N], f8)
        for i in range(2):
            nc.sync.dma_start(out=xb8[:, i, 0:NQ], in_=xb8_d[:, N * i:N * i + NQ])

        # ---- group stats -> per-channel A (f32) and B (bf16) ----
        # Sum x^2 = diag(x^T x): mask out the diagonal, then column-sum it
        # back to [128,1] with a N=1 matmul (diag matrix -> col sums = diag).
        ones_col = consts.tile([128, 2], f32)
        nc.vector.memset(ones_col, 1.0)
        ones_colr = consts.tile([128, 2], f32r)
        nc.vector.tensor_copy(ones_colr, ones_col)
        stile = work.tile([128, 2, 2], f32, name="stile")  # [:, h, (sx, sxx)]
        for h in range(2):
            nc.vector.tensor_copy(stile[:, h, 0:1], xx[h][:, 128:129])
            scr = work.tile([128, 128], f32r, name="scr", tag="scr", bufs=2)
            nc.vector.tensor_mul(scr, xx[h][:, 0:128], imask)
            sxpool = pw if h == 0 else pr
            sxp = sxpool.tile([128, 2], f32, name="sxp", tag="pw" if h == 0 else "r")
            nc.tensor.matmul(sxp, lhsT=scr, rhs=ones_colr, start=True, stop=True)
            nc.vector.tensor_copy(stile[:, h, 1:2], sxp[:, 0:1])
        gp = pacc.tile([GROUPS, 2], f32, tag="u")
        for h in range(2):
            nc.tensor.matmul(gp, lhsT=gmaskT[h], rhs=stile[:, h, :],
                             start=(h == 0), stop=(h == 1))
        # gmaskT is host-scaled by 1/(32N), so gp = (mean, E[x^2]) directly.
        gsb = work.tile([GROUPS, 2], f32, name="gsb")
        nc.vector.tensor_copy(gsb, gp)
        negvar = work.tile([GROUPS, 1], f32, name="negvar")
        nc.vector.scalar_tensor_tensor(out=negvar, in0=gsb[:, 0:1],
                                       scalar=gsb[:, 0:1], in1=gsb[:, 1:2],
                                       op0=MULT, op1=mybir.AluOpType.subtract)
        gsd = work.tile([GROUPS, 1], f32, name="gsd")
        nc.scalar.activation(out=gsd, in_=negvar, func=AF.Sqrt, bias=eps8,
                             scale=-1.0)
        # preload the Exp act table now (the Sqrt above was ACT's last
        # non-Exp op; loading here keeps the main loop table-stable)
        dummye = consts.tile([1, 1], f32)
        nc.scalar.activation(out=dummye, in_=gsd[0:1, :], func=AF.Exp,
                             scale=1.0, bias=nbias[0:1, :])
        nc.vector.reciprocal(out=gsb[:, 1:2], in_=gsd)

        # B2 holds -B = mean*A - beta (sign fixed up at the consumers)
        A2 = consts.tile([128, 2], f32)
        B2 = consts.tile([128, 2], bf16)
        B2f = work.tile([128, 2], f32, name="B2f")
        for i in range(2):
            gbp = pw.tile([128, 2], f32, name="gbp", tag="pw")
            nc.tensor.matmul(gbp, lhsT=gmask8[:, 128 * i:128 * (i + 1)],
                             rhs=gsb, start=True, stop=True)
            nc.vector.tensor_mul(A2[:, i:i + 1], gamma2[:, i:i + 1], gbp[:, 1:2])
            nc.vector.scalar_tensor_tensor(out=B2f[:, i:i + 1], in0=gbp[:, 0:1],
                                           scalar=A2[:, i:i + 1],
                                           in1=beta2[:, i:i + 1], op0=MULT,
                                           op1=mybir.AluOpType.subtract)
        nc.vector.tensor_copy(B2, B2f)
        if DBG:
            dt1 = work.tile([128, 4], f32, name="dt1")
            nc.vector.tensor_copy(dt1, stile)
            nc.sync.dma_start(out=dbg_stile, in_=dt1)
            dt2 = work.tile([128, 4], f32, name="dt2")
            nc.vector.tensor_copy(dt2[:, 0:2], A2)
            nc.vector.tensor_copy(dt2[:, 2:4], B2f)
            nc.sync.dma_start(out=dbg_a2, in_=dt2)

        # ---- fold A into G^T -> fp8 (contraction-side fold for q') ----
        ga8 = consts.tile([128, 2, C], f8)
        for i in range(2):
            nc.vector.tensor_scalar_mul(ga8[:, i, :], gt[:, i, :], A2[:, i:i + 1])

        # ---- delta coefficient: coef = A o (G B + Wk^T bq) ----
        # Folded into the q' drain below: q8 = A*qp + coef makes the single
        # S matmul compute S + delta[m] directly (delta const over n).
        gbv = pacc.tile([128, 2], f32, name="gbv", tag="u")
        for ch in range(2):
            for i in range(2):
                nc.tensor.matmul(gbv[:, ch:ch + 1],
                                 lhsT=gt[:, i, ch * 128:(ch + 1) * 128],
                                 rhs=B2[:, i:i + 1], start=(i == 0), stop=(i == 1))
        coef = work.tile([128, 2], f32, name="coef")
        nc.vector.tensor_sub(coef, wtld2, gbv)     # gbv = -G B
        nc.vector.tensor_mul(coef, coef, A2)

        # ---- q'[c, n] = A[c] * sum_c' (G[c,c'] A[c']) x_q[c', n] + coef[c] ----
        q8 = big.tile([128, 2, NQ], f8)

        def emit_q(qc):
            for ch in range(2):
                ns = slice(qc * 512, (qc + 1) * 512)
                if qc == 0:
                    qp = psp.tile([128, 512], f32, name="qp", tag="sp")
                else:
                    qp = pw.tile([128, 512], f32, name="qp", tag="pw")
                nc.tensor.matmul(qp, lhsT=ga8[:, :, ch * 128:(ch + 1) * 128],
                                 rhs=xb8[:, :, ns], start=True, stop=True,
                                 perf_mode=DR)
                nc.vector.tensor_scalar(out=q8[:, ch, ns], in0=qp,
                                        scalar1=A2[:, ch:ch + 1],
                                        scalar2=coef[:, ch:ch + 1],
                                        op0=MULT, op1=ADD)

        emit_q(0)

        if DBG:
            dt3 = work.tile([128, 2, NQ], f32, name="dt3", tag="dbgbig")
            nc.vector.tensor_copy(dt3, q8)
            nc.sync.dma_start(out=dbg_q8, in_=dt3)
            dt4 = work.tile([128, 2, 512], f32, name="dt4", tag="dbgbig2")
            nc.vector.tensor_copy(dt4, coef8b)
            nc.sync.dma_start(out=dbg_coef, in_=dt4)

        # ---- rest of the loads (xb8 keys, weights for biases, xq last) ----
        for i in range(2):
            nc.sync.dma_start(out=xb8[:, i, NQ:N],
                              in_=xb8_d[:, N * i + NQ:N * (i + 1)])
        dxq = big.tile([128, 2, NQ], bf16)
        nc.sync.dma_start(out=dxq, in_=dxq_d)
        wall = consts.tile([128, 2, 2 * C], bf16)   # [wv, wp]
        nc.sync.dma_start(out=wall, in_=wall_d)
        pvt = consts.tile([128, 2, C], bf16)
        nc.sync.dma_start(out=pvt, in_=pvt_d)

        # ---- fold A into Pv^T -> fp8 (fused Wp @ Wv projection) ----
        pva8 = consts.tile([128, 2, C], f8)
        for i in range(2):
            nc.vector.tensor_scalar_mul(pva8[:, i, :], pvt[:, i, :],
                                        A2[:, i:i + 1])

        # ---- cv = Wv B + bv (bf16); bpe2 = bp + Wp cv; xqb = xq + bpe2 ----
        cvf = work.tile([128, 2], f32, name="cvf")
        for ot in range(2):
            cvpool = pw if ot == 0 else pr
            p = cvpool.tile([128, 1], f32, name="cvp",
                            tag="pw" if ot == 0 else "r")
            for i in range(2):
                nc.tensor.matmul(p, lhsT=wall[:, i, 128 * ot:128 * (ot + 1)],
                                 rhs=B2[:, i:i + 1], start=(i == 0), stop=(i == 1))
            nc.vector.tensor_sub(cvf[:, ot:ot + 1], bv2[:, ot:ot + 1], p)
        cv = consts.tile([128, 2], bf16)
        nc.vector.tensor_copy(cv, cvf)
        bpe2 = work.tile([128, 2], f32, name="bpe2")
        for ot in range(2):
            bppool = pw if ot == 0 else pr
            p2 = bppool.tile([128, 1], f32, name="bpp",
                             tag="pw" if ot == 0 else "r")
            for i in range(2):
                nc.tensor.matmul(p2, lhsT=wall[:, i, C + 128 * ot:C + 128 * (ot + 1)],
                                 rhs=cv[:, i:i + 1], start=(i == 0), stop=(i == 1))
            nc.vector.tensor_scalar_add(bpe2[:, ot:ot + 1], p2, bp2[:, ot:ot + 1])
        # residual x_q reconstructed as fp8(x) + bf16 quantization remainder
        xqb = big.tile([128, 2, NQ], f32)
        for ot in range(2):
            nc.gpsimd.tensor_add(xqb[:, ot, :], xb8[:, ot, 0:NQ], dxq[:, ot, :])
            nc.gpsimd.tensor_scalar_add(xqb[:, ot, :], xqb[:, ot, :],
                                        bpe2[:, ot:ot + 1])

        # ---- attention: S + delta -> exp -> r, u0, u1 -> scale -> project ----
        u8 = big.tile([128, 2, NQ], f8)
        for nch in range(2):
            ns = slice(nch * 512, (nch + 1) * 512)
            rp = pr.tile([128, 512], f32, name="rp", tag="r")
            up = [pacc.tile([128, 512], f32, name=f"up{h}", tag="u")
                  for h in range(2)]
            for s in range(16):
                if nch == 0 and s == 2:
                    emit_q(1)
                sp = psp.tile([128, 1024], f32, name="sp", tag="sp")
                for h in range(2):
                    ms = slice((2 * s + h) * 128, (2 * s + h + 1) * 128)
                    hs = slice(512 * h, 512 * (h + 1))
                    nc.tensor.matmul(sp[:, hs], lhsT=xb8[:, :, ms],
                                     rhs=q8[:, :, ns], start=True, stop=True,
                                     perf_mode=DR)
                e = epool.tile([128, 2, 512], f8, name="e", tag="e")
                if DBG and nch == 0 and s == 0:
                    dt5 = work.tile([128, 1024], f32, name="dt5", tag="dbgbig3")
                    nc.vector.tensor_copy(dt5, sp)
                    nc.sync.dma_start(out=dbg_sp, in_=dt5)
                nc.scalar.activation(out=e, in_=sp, func=AF.Exp,
                                     scale=1.0 / 16.0, bias=nbias)
                nc.tensor.matmul(rp, lhsT=ones8, rhs=e,
                                 start=(s == 0), stop=(s == 15), perf_mode=DR)
                for h in range(2):
                    nc.tensor.matmul(
                        up[h],
                        lhsT=xt8[:, 2 * s:2 * s + 2, 136 * h:136 * h + 128],
                        rhs=e, start=(s == 0), stop=(s == 15), perf_mode=DR)
            if DBG and nch == 0:
                dt6 = work.tile([128, 512], f32, name="dt6", tag="dbgbig4")
                nc.vector.tensor_copy(dt6, rp)
                nc.sync.dma_start(out=dbg_rp, in_=dt6)
            rb = work.tile([128, 512], f32, name="rb", tag="rb", bufs=2)
            nc.vector.reciprocal(out=rb, in_=rp)
            for h in range(2):
                nc.vector.tensor_mul(u8[:, h, ns], up[h], rb)

            if DBG and nch == 1:
                dt7 = work.tile([128, 2, NQ], f32, name="dt7", tag="dbgbig5")
                nc.vector.tensor_copy(dt7, u8)
                nc.sync.dma_start(out=dbg_u8, in_=dt7)
                dt8 = work.tile([1, C], f32, name="dt8", tag="dbgbig6")
                nc.vector.tensor_copy(dt8, bpeT)
                nc.sync.dma_start(out=dbg_bpe, in_=dt8)
            # fused projection; out = pp/64 + (xq + bpe) in one stt
            for ot in range(2):
                if nch == 1:
                    pp = psp.tile([128, 512], f32, name="pp", tag="sp")
                else:
                    pp = pw.tile([128, 512], f32, name="pp", tag="pw")
                nc.tensor.matmul(pp, lhsT=pva8[:, :, ot * 128:(ot + 1) * 128],
                                 rhs=u8[:, :, ns], start=True, stop=True,
                                 perf_mode=DR)
                ot_t = work.tile([128, 512], f32, name="ot_t", tag="ot_t")
                nc.vector.scalar_tensor_tensor(out=ot_t, in0=pp,
                                               scalar=1.0 / 64.0,
                                               in1=xqb[:, ot, ns], op0=MULT,
                                               op1=ADD)
                nc.sync.dma_start(out=out_d[:, NQ * ot + 512 * nch:
                                            NQ * ot + 512 * (nch + 1)],
                                  in_=ot_t)

    nc.compile()
    return nc


def _get_nc():
    key = "nc"
    if key not in _CACHE:
        _CACHE[key] = _build_nc()
    return _CACHE[key]


def _host_inputs(x, gamma, beta, Wq, bq, Wk, bk, Wv, bv, Wp, bp):
    import ml_dtypes
    f8 = ml_dtypes.float8_e4m3
    bf = ml_dtypes.bfloat16

    x = np.asarray(x, np.float32)
    xf = np.ascontiguousarray(x.reshape(2, C, N))
    gamma = np.asarray(gamma, np.float32)
    beta = np.asarray(beta, np.float32)
    Wq, Wk, Wv, Wp = [np.asarray(W, np.float32) for W in (Wq, Wk, Wv, Wp)]
    bq, bv, bp = [np.asarray(v, np.float32) for v in (bq, bv, bp)]

    # host-fused matrices: G = Wk^T Wq, Pv = Wp @ Wv
    G = Wk.T @ Wq                                             # [c, c']
    Pv = Wp @ Wv                                              # [o, c]
    # gt[p, i, c] = G[c, i*128+p]  (contraction over c' = i*128+p)
    gt = np.ascontiguousarray(
        G.T.reshape(2, 128, C).transpose(1, 0, 2).reshape(128, 2 * C)
    ).astype(bf)
    # pvt[p, i, o] = Pv[o, i*128+p]; the r-matmul's 1/64 stationary makes
    # u8 = 64*u/r (fp8 normal range), repaid by the epilogue's /64.
    pvt = np.ascontiguousarray(
        Pv.T.reshape(2, 128, C).transpose(1, 0, 2).reshape(128, 2 * C)
    ).astype(bf)
    # wall[p, i, (wv|wp), o] = W[o, i*128+p]
    wall = np.stack([Wv.T, Wp.T], axis=1)                     # [c, 2, o]
    wall = wall.reshape(2, 128, 2, C).transpose(1, 0, 2, 3)
    wall = np.ascontiguousarray(wall.reshape(128, 2 * 2 * C)).astype(bf)

    small = np.zeros((128, 26), np.float32)
    small[:, 0:2] = gamma.reshape(2, 128).T
    small[:, 2:4] = beta.reshape(2, 128).T
    small[:, 4:6] = bv.reshape(2, 128).T
    small[:, 6:8] = (Wk.T @ bq).reshape(2, 128).T
    small[:, 8:10] = bp.reshape(2, 128).T
    cids = np.arange(C)
    gm = np.zeros((C, GROUPS), np.float32)
    gm[cids, cids // GSIZE] = 1.0                             # [c, g]
    gmT = gm.reshape(2, 128, GROUPS).transpose(1, 0, 2)       # [p, i, g]
    small[:, 10:18] = gmT[:, 0, :] / (GSIZE * N)
    small[:, 18:26] = gmT[:, 1, :] / (GSIZE * N)
    gmask8 = np.ascontiguousarray(gm.T)                       # [g, c]
    imask = np.eye(128, dtype=np.float32).astype(bf)

    in_maps = []
    for core in range(8):
        b, j = divmod(core, 4)
        xrot = np.roll(xf[b], -j * NQ, axis=1)                # [C, N]
        x8 = xrot.astype(f8)
        xb8 = np.ascontiguousarray(
            x8.reshape(2, 128, N).transpose(1, 0, 2).reshape(128, 2 * N))
        dxq = (xrot[:, :NQ] - x8[:, :NQ].astype(np.float32))
        dxq = np.ascontiguousarray(
            dxq.reshape(2, 128, NQ).transpose(1, 0, 2).reshape(128, 2 * NQ)
        ).astype(bf)
        # xt8: position-major with ones cols: per n: [c0..c127, 1, c128.., 1]
        xt = x8.astype(np.float32).T                          # [n, c] quantized
        arr = np.zeros((N, 272), np.float32)
        arr[:, 0:128] = xt[:, 0:128]
        arr[:, 128] = 1.0
        arr[:, 136:264] = xt[:, 128:256]
        arr[:, 264] = 1.0
        xt8 = np.ascontiguousarray(
            arr.reshape(32, 128, 272).transpose(1, 0, 2).reshape(128, 32 * 272)
        ).astype(f8)
        in_maps.append({
            "xt8": xt8, "xb8": xb8, "gt": gt, "pvt": pvt, "wall": wall,
            "small": small, "gmask8": gmask8, "imask": imask,
            "dxq": dxq,
        })
    return in_maps


def kernel(x, gamma, beta, Wq, bq, Wk, bk, Wv, bv, Wp, bp):
    from concourse.bass_utils import run_bass_kernel_spmd
    global LAST_RESULTS

    orig_shape = np.asarray(x).shape
    in_maps = _host_inputs(x, gamma, beta, Wq, bq, Wk, bk, Wv, bv, Wp, bp)
    nc = _get_nc()

    trace = os.environ.get("BASSK_TRACE", "0") == "1"
    res = run_bass_kernel_spmd(nc, in_maps, core_ids=list(range(8)), trace=trace)
    LAST_RESULTS = res

    out = np.empty((2, C, N), np.float32)
    for core in range(8):
        b, j = divmod(core, 4)
        o = res.results[core]["out"]                          # [128, 2*NQ]
        o = o.reshape(128, 2, NQ).transpose(1, 0, 2).reshape(C, NQ)
        out[b][:, j * NQ:(j + 1) * NQ] = o
    return out.reshape(orig_shape)


# revision 34
# speedup vs baseline: 2.1418x; 1.0002x over previous
"""AttentionBlock3D (GroupNorm + single-head self-attention + residual) on 8 TRN2 cores.

Sharding: core = (batch b in {0,1}) x (1024-row slice of the 4096 attention rows).
Each core computes its batch's GroupNorm stats (cheap, on the PE) and
attention + output projection for its own 1024 query rows. No collectives.
The host ROTATES each core's x copy so that its query rows are always
columns 0..1024 (attention is permutation-invariant over keys).

fp8 DoubleRow pipeline with K and V eliminated:
  - x ships twice in fp8 e4m3: xb8 [128, 2, N] (channel-major) and
    xt8 [128, 32, 258] (position-major with built-in ones columns).
  - GroupNorm stats on the PE: x^T x accumulated per channel half; the ones
    column gives Sum x, the diagonal (mask reduce) gives Sum x^2.
  - S = hn^T G hn_q with G = Wk^T Wq (host): q' = (G.A-folded)^T x_q scaled
    by A, S = x^T q' via DoubleRow with xb8 stationary — no K tensor.
    Per-key bias delta[m] = (A o (G B + Wk^T bq))^T x[:,m] enters S as one
    extra DoubleRow matmul with constant-column rhs (coef broadcast).
  - u = x^T E via DoubleRow with xt8 stationary — no V tensor; the output
    projection fuses Pv = Wp @ Wv (host) with A folded on device:
    out = x_q + bpe + (Pv.A)^T u / r,  bpe = bp + Wp cv, cv = Wv B + bv.
  - E = exp(S/16 - 2) on ACT in [128,1024] tiles (e^-2 cancels against r).
    ACT exp is the bottleneck engine (~4.2M elements/core).
"""

import os
import numpy as np
from contextlib import ExitStack

C = 256          # channels
N = 4096         # spatial positions (16*16*16)
NQ = 1024        # query rows per core
GROUPS = 8
GSIZE = C // GROUPS
EPS = 1e-5

_CACHE = {}
LAST_RESULTS = None  # test harness can inspect trace results


def _build_nc():
    import concourse.bacc as bacc
    import concourse.tile as tile
    from concourse import mybir

    f32 = mybir.dt.float32
    f32r = mybir.dt.float32r
    bf16 = mybir.dt.bfloat16
    f8 = mybir.dt.float8e4
    AF = mybir.ActivationFunctionType
    DR = mybir.MatmulPerfMode.DoubleRow
    ADD = mybir.AluOpType.add
    MULT = mybir.AluOpType.mult

    nc = bacc.Bacc("TRN2", target_bir_lowering=False, debug=False,
                   enable_asserts=False)

    # ---- DRAM I/O (per-core) ----
    xt8_d = nc.dram_tensor("xt8", [128, 32 * 272], f8, kind="ExternalInput").ap()
    xb8_d = nc.dram_tensor("xb8", [128, 2 * N], f8, kind="ExternalInput").ap()
    gt_d = nc.dram_tensor("gt", [128, 2 * C], bf16, kind="ExternalInput").ap()
    pvt_d = nc.dram_tensor("pvt", [128, 2 * C], bf16, kind="ExternalInput").ap()
    wall_d = nc.dram_tensor("wall", [128, 2 * 2 * C], bf16, kind="ExternalInput").ap()
    small_d = nc.dram_tensor("small", [128, 26], f32, kind="ExternalInput").ap()
    gmask8_d = nc.dram_tensor("gmask8", [GROUPS, C], f32, kind="ExternalInput").ap()
    imask_d = nc.dram_tensor("imask", [128, 128], bf16, kind="ExternalInput").ap()
    dxq_d = nc.dram_tensor("dxq", [128, 2 * NQ], bf16, kind="ExternalInput").ap()
    out_d = nc.dram_tensor("out", [128, 2 * NQ], f32, kind="ExternalOutput").ap()
    DBG = os.environ.get("BASSK_DBG", "0") == "1"
    if DBG:
        dbg_stile = nc.dram_tensor("dbg_stile", [128, 4], f32, kind="ExternalOutput").ap()
        dbg_a2 = nc.dram_tensor("dbg_a2", [128, 4], f32, kind="ExternalOutput").ap()
        dbg_q8 = nc.dram_tensor("dbg_q8", [128, 2 * NQ], f32, kind="ExternalOutput").ap()
        dbg_coef = nc.dram_tensor("dbg_coef", [128, 2 * 512], f32, kind="ExternalOutput").ap()
        dbg_sp = nc.dram_tensor("dbg_sp", [128, 1024], f32, kind="ExternalOutput").ap()
        dbg_rp = nc.dram_tensor("dbg_rp", [128, 512], f32, kind="ExternalOutput").ap()
        dbg_u8 = nc.dram_tensor("dbg_u8", [128, 2 * NQ], f32, kind="ExternalOutput").ap()
        dbg_bpe = nc.dram_tensor("dbg_bpe", [1, C], f32, kind="ExternalOutput").ap()

    with tile.TileContext(nc) as tc, ExitStack() as ctx:
        big = ctx.enter_context(tc.tile_pool(name="big", bufs=1))
        consts = ctx.enter_context(tc.tile_pool(name="consts", bufs=1))
        work = ctx.enter_context(tc.tile_pool(name="work", bufs=3))
        epool = ctx.enter_context(tc.tile_pool(name="epool", bufs=3))
        # PSUM banks: psp 2x2 + pacc 2 + pr 1 + pw 1 = 8
        psp = ctx.enter_context(tc.tile_pool(name="psp", bufs=2, space="PSUM"))
        pacc = ctx.enter_context(tc.tile_pool(name="pacc", bufs=2, space="PSUM"))
        pr = ctx.enter_context(tc.tile_pool(name="pr", bufs=1, space="PSUM"))
        pw = ctx.enter_context(tc.tile_pool(name="pw", bufs=1, space="PSUM"))

        # ---- constants (before the big loads) ----
        # r-matmul stationary = 1/64 so the reciprocal yields 64/r; the /64
        # is repaid in the epilogue stt. Keeps u8 = 64*u/r in fp8's sweet spot.
        ones8 = consts.tile([128, 2, 128], f8)
        nc.vector.memset(ones8, 1.0 / 64.0)
        nbias = consts.tile([128, 1], f32)
        nc.vector.memset(nbias, -3.5)
        # eps8 = Sqrt(EPS^2) on ACT: forces the Sqrt act-table load at t~0
        eps_sq = consts.tile([GROUPS, 1], f32)
        nc.vector.memset(eps_sq, EPS * EPS)
        eps8 = consts.tile([GROUPS, 1], f32)
        nc.scalar.activation(out=eps8, in_=eps_sq, func=AF.Sqrt, scale=1.0)

        # ---- load xt8 (stats + u path) chunked; stats matmuls interleave ----
        xt8 = big.tile([128, 32, 272], f8)
        for ch in range(4):
            nc.sync.dma_start(out=xt8[:, 8 * ch:8 * ch + 8, :],
                              in_=xt8_d[:, 8 * 272 * ch:8 * 272 * (ch + 1)])
        # xx[h] accumulates x^T x for channel half h ([128,129]: 128 cols of
        # the x^T x block + ones column giving Sum x). Uses the idle sp slots.
        xx = []
        for h in range(2):
            t = psp.tile([128, 129], f32, name=f"xx{h}", tag="sp")
            xx.append(t)
        for s in range(16):
            for h in range(2):
                nc.tensor.matmul(
                    xx[h],
                    lhsT=xt8[:, 2 * s:2 * s + 2, 136 * h:136 * h + 128],
                    rhs=xt8[:, 2 * s:2 * s + 2, 136 * h:136 * h + 129],
                    start=(s == 0), stop=(s == 15), perf_mode=DR)

        # ---- smalls + G^T + xb8 query cols (early, for q') ----
        small_sb = consts.tile([128, 26], f32)
        nc.sync.dma_start(out=small_sb, in_=small_d)
        imask = consts.tile([128, 128], bf16)
        nc.sync.dma_start(out=imask, in_=imask_d)
        gmask8 = consts.tile([GROUPS, C], f32)
        nc.sync.dma_start(out=gmask8, in_=gmask8_d)
        gamma2 = small_sb[:, 0:2]
        beta2 = small_sb[:, 2:4]
        bv2 = small_sb[:, 4:6]
        wtld2 = small_sb[:, 6:8]          # Wk^T bq (host)
        bp2 = small_sb[:, 8:10]
        gmaskT = [small_sb[:, 10 + 8 * i:18 + 8 * i] for i in range(2)]  # pre-scaled 1/(32N)
        gt = consts.tile([128, 2, C], bf16)
        nc.sync.dma_start(out=gt, in_=gt_d)
        xb8 = big.tile([128, 2, N], f8)
        for i in range(2):
            nc.sync.dma_start(out=xb8[:, i, 0:NQ], in_=xb8_d[:, N * i:N * i + NQ])

        # ---- group stats -> per-channel A (f32) and B (bf16) ----
        # Sum x^2 = diag(x^T x): mask out the diagonal, then column-sum it
        # back to [128,1] with a N=1 matmul (diag matrix -> col sums = diag).
        ones_col = consts.tile([128, 2], f32)
        nc.vector.memset(ones_col, 1.0)
        ones_colr = consts.tile([128, 2], f32r)
        nc.vector.tensor_copy(ones_colr, ones_col)
        stile = work.tile([128, 2, 2], f32, name="stile")  # [:, h, (sx, sxx)]
        for h in range(2):
            nc.vector.tensor_copy(stile[:, h, 0:1], xx[h][:, 128:129])
            scr = work.tile([128, 128], f32r, name="scr", tag="scr", bufs=2)
            nc.vector.tensor_mul(scr, xx[h][:, 0:128], imask)
            sxpool = pw if h == 0 else pr
            sxp = sxpool.tile([128, 2], f32, name="sxp", tag="pw" if h == 0 else "r")
            nc.tensor.matmul(sxp, lhsT=scr, rhs=ones_colr, start=True, stop=True)
            nc.vector.tensor_copy(stile[:, h, 1:2], sxp[:, 0:1])
        gp = pacc.tile([GROUPS, 2], f32, tag="u")
        for h in range(2):
            nc.tensor.matmul(gp, lhsT=gmaskT[h], rhs=stile[:, h, :],
                             start=(h == 0), stop=(h == 1))
        # gmaskT is host-scaled by 1/(32N), so gp = (mean, E[x^2]) directly.
        gsb = work.tile([GROUPS, 2], f32, name="gsb")
        nc.vector.tensor_copy(gsb, gp)
        negvar = work.tile([GROUPS, 1], f32, name="negvar")
        nc.vector.scalar_tensor_tensor(out=negvar, in0=gsb[:, 0:1],
                                       scalar=gsb[:, 0:1], in1=gsb[:, 1:2],
                                       op0=MULT, op1=mybir.AluOpType.subtract)
        gsd = work.tile([GROUPS, 1], f32, name="gsd")
        nc.scalar.activation(out=gsd, in_=negvar, func=AF.Sqrt, bias=eps8,
                             scale=-1.0)
        # preload the Exp act table now (the Sqrt above was ACT's last
        # non-Exp op; loading here keeps the main loop table-stable)
        dummye = consts.tile([1, 1], f32)
        nc.scalar.activation(out=dummye, in_=gsd[0:1, :], func=AF.Exp,
                             scale=1.0, bias=nbias[0:1, :])
        nc.vector.reciprocal(out=gsb[:, 1:2], in_=gsd)

        # B2 holds -B = mean*A - beta (sign fixed up at the consumers)
        A2 = consts.tile([128, 2], f32)
        B2 = consts.tile([128, 2], bf16)
        B2f = work.tile([128, 2], f32, name="B2f")
        for i in range(2):
            gbp = pw.tile([128, 2], f32, name="gbp", tag="pw")
            nc.tensor.matmul(gbp, lhsT=gmask8[:, 128 * i:128 * (i + 1)],
                             rhs=gsb, start=True, stop=True)
            nc.vector.tensor_mul(A2[:, i:i + 1], gamma2[:, i:i + 1], gbp[:, 1:2])
            nc.vector.scalar_tensor_tensor(out=B2f[:, i:i + 1], in0=gbp[:, 0:1],
                                           scalar=A2[:, i:i + 1],
                                           in1=beta2[:, i:i + 1], op0=MULT,
                                           op1=mybir.AluOpType.subtract)
        nc.vector.tensor_copy(B2, B2f)
        if DBG:
            dt1 = work.tile([128, 4], f32, name="dt1")
            nc.vector.tensor_copy(dt1, stile)
            nc.sync.dma_start(out=dbg_stile, in_=dt1)
            dt2 = work.tile([128, 4], f32, name="dt2")
            nc.vector.tensor_copy(dt2[:, 0:2], A2)
            nc.vector.tensor_copy(dt2[:, 2:4], B2f)
            nc.sync.dma_start(out=dbg_a2, in_=dt2)

        # ---- fold A into G^T -> fp8 (contraction-side fold for q') ----
        ga8 = consts.tile([128, 2, C], f8)
        for i in range(2):
            nc.vector.tensor_scalar_mul(ga8[:, i, :], gt[:, i, :], A2[:, i:i + 1])

        # ---- delta coefficient: coef = A o (G B + Wk^T bq) ----
        # Folded into the q' drain below: q8 = A*qp + coef makes the single
        # S matmul compute S + delta[m] directly (delta const over n).
        gbv = pacc.tile([128, 2], f32, name="gbv", tag="u")
        for ch in range(2):
            for i in range(2):
                nc.tensor.matmul(gbv[:, ch:ch + 1],
                                 lhsT=gt[:, i, ch * 128:(ch + 1) * 128],
                                 rhs=B2[:, i:i + 1], start=(i == 0), stop=(i == 1))
        coef = work.tile([128, 2], f32, name="coef")
        nc.vector.tensor_sub(coef, wtld2, gbv)     # gbv = -G B
        nc.vector.tensor_mul(coef, coef, A2)

        # ---- q'[c, n] = A[c] * sum_c' (G[c,c'] A[c']) x_q[c', n] + coef[c] ----
        q8 = big.tile([128, 2, NQ], f8)

        def emit_q(qc):
            for ch in range(2):
                ns = slice(qc * 512, (qc + 1) * 512)
                if qc == 0:
                    qp = psp.tile([128, 512], f32, name="qp", tag="sp")
                else:
                    qp = pw.tile([128, 512], f32, name="qp", tag="pw")
                nc.tensor.matmul(qp, lhsT=ga8[:, :, ch * 128:(ch + 1) * 128],
                                 rhs=xb8[:, :, ns], start=True, stop=True,
                                 perf_mode=DR)
                if qc == 0 and ch == 1:
                    # ACT is idle pre-stream; halves the head's drain chain
                    nc.scalar.activation(out=q8[:, ch, ns], in_=qp,
                                         func=AF.Identity,
                                         scale=A2[:, ch:ch + 1],
                                         bias=coef[:, ch:ch + 1])
                else:
                    nc.vector.tensor_scalar(out=q8[:, ch, ns], in0=qp,
                                            scalar1=A2[:, ch:ch + 1],
                                            scalar2=coef[:, ch:ch + 1],
                                            op0=MULT, op1=ADD)

        emit_q(0)

        if DBG:
            dt3 = work.tile([128, 2, NQ], f32, name="dt3", tag="dbgbig")
            nc.vector.tensor_copy(dt3, q8)
            nc.sync.dma_start(out=dbg_q8, in_=dt3)
            dt4 = work.tile([128, 2, 512], f32, name="dt4", tag="dbgbig2")
            nc.vector.tensor_copy(dt4, coef8b)
            nc.sync.dma_start(out=dbg_coef, in_=dt4)

        # ---- rest of the loads (xb8 keys, weights for biases, xq last) ----
        for i in range(2):
            nc.sync.dma_start(out=xb8[:, i, NQ:N],
                              in_=xb8_d[:, N * i + NQ:N * (i + 1)])
        dxq = big.tile([128, 2, NQ], bf16)
        nc.sync.dma_start(out=dxq, in_=dxq_d)
        wall = consts.tile([128, 2, 2 * C], bf16)   # [wv, wp]
        nc.sync.dma_start(out=wall, in_=wall_d)
        pvt = consts.tile([128, 2, C], bf16)
        nc.sync.dma_start(out=pvt, in_=pvt_d)

        # ---- fold A into Pv^T -> fp8 (fused Wp @ Wv projection) ----
        pva8 = consts.tile([128, 2, C], f8)
        for i in range(2):
            nc.vector.tensor_scalar_mul(pva8[:, i, :], pvt[:, i, :],
                                        A2[:, i:i + 1])

        # ---- cv = Wv B + bv (bf16); bpe2 = bp + Wp cv; xqb = xq + bpe2 ----
        cvf = work.tile([128, 2], f32, name="cvf")
        for ot in range(2):
            cvpool = pw if ot == 0 else pr
            p = cvpool.tile([128, 1], f32, name="cvp",
                            tag="pw" if ot == 0 else "r")
            for i in range(2):
                nc.tensor.matmul(p, lhsT=wall[:, i, 128 * ot:128 * (ot + 1)],
                                 rhs=B2[:, i:i + 1], start=(i == 0), stop=(i == 1))
            nc.vector.tensor_sub(cvf[:, ot:ot + 1], bv2[:, ot:ot + 1], p)
        cv = consts.tile([128, 2], bf16)
        nc.vector.tensor_copy(cv, cvf)
        bpe2 = work.tile([128, 2], f32, name="bpe2")
        for ot in range(2):
            bppool = pw if ot == 0 else pr
            p2 = bppool.tile([128, 1], f32, name="bpp",
                             tag="pw" if ot == 0 else "r")
            for i in range(2):
                nc.tensor.matmul(p2, lhsT=wall[:, i, C + 128 * ot:C + 128 * (ot + 1)],
                                 rhs=cv[:, i:i + 1], start=(i == 0), stop=(i == 1))
            nc.vector.tensor_scalar_add(bpe2[:, ot:ot + 1], p2, bp2[:, ot:ot + 1])
        # residual x_q reconstructed as fp8(x) + bf16 quantization remainder
        xqb = big.tile([128, 2, NQ], f32)
        for ot in range(2):
            nc.gpsimd.tensor_add(xqb[:, ot, :], xb8[:, ot, 0:NQ], dxq[:, ot, :])
            nc.gpsimd.tensor_scalar_add(xqb[:, ot, :], xqb[:, ot, :],
                                        bpe2[:, ot:ot + 1])

        # ---- attention: S + delta -> exp -> r, u0, u1 -> scale -> project ----
        u8 = big.tile([128, 2, NQ], f8)
        for nch in range(2):
            ns = slice(nch * 512, (nch + 1) * 512)
            rp = pr.tile([128, 512], f32, name="rp", tag="r")
            up = [pacc.tile([128, 512], f32, name=f"up{h}", tag="u")
                  for h in range(2)]
            for s in range(16):
                if nch == 0 and s == 2:
                    emit_q(1)
                sp = psp.tile([128, 1024], f32, name="sp", tag="sp")
                for h in range(2):
                    ms = slice((2 * s + h) * 128, (2 * s + h + 1) * 128)
                    hs = slice(512 * h, 512 * (h + 1))
                    nc.tensor.matmul(sp[:, hs], lhsT=xb8[:, :, ms],
                                     rhs=q8[:, :, ns], start=True, stop=True,
                                     perf_mode=DR)
                e = epool.tile([128, 2, 512], f8, name="e", tag="e")
                if DBG and nch == 0 and s == 0:
                    dt5 = work.tile([128, 1024], f32, name="dt5", tag="dbgbig3")
                    nc.vector.tensor_copy(dt5, sp)
                    nc.sync.dma_start(out=dbg_sp, in_=dt5)
                nc.scalar.activation(out=e, in_=sp, func=AF.Exp,
                                     scale=1.0 / 16.0, bias=nbias)
                nc.tensor.matmul(rp, lhsT=ones8, rhs=e,
                                 start=(s == 0), stop=(s == 15), perf_mode=DR)
                for h in range(2):
                    nc.tensor.matmul(
                        up[h],
                        lhsT=xt8[:, 2 * s:2 * s + 2, 136 * h:136 * h + 128],
                        rhs=e, start=(s == 0), stop=(s == 15), perf_mode=DR)
            if DBG and nch == 0:
                dt6 = work.tile([128, 512], f32, name="dt6", tag="dbgbig4")
                nc.vector.tensor_copy(dt6, rp)
                nc.sync.dma_start(out=dbg_rp, in_=dt6)
            rb = work.tile([128, 512], f32, name="rb", tag="rb", bufs=2)
            nc.vector.reciprocal(out=rb, in_=rp)
            for h in range(2):
                nc.vector.tensor_mul(u8[:, h, ns], up[h], rb)

            if DBG and nch == 1:
                dt7 = work.tile([128, 2, NQ], f32, name="dt7", tag="dbgbig5")
                nc.vector.tensor_copy(dt7, u8)
                nc.sync.dma_start(out=dbg_u8, in_=dt7)
                dt8 = work.tile([1, C], f32, name="dt8", tag="dbgbig6")
                nc.vector.tensor_copy(dt8, bpeT)
                nc.sync.dma_start(out=dbg_bpe, in_=dt8)
            # fused projection; out = pp/64 + (xq + bpe) in one stt
            for ot in range(2):
                if nch == 1:
                    pp = psp.tile([128, 512], f32, name="pp", tag="sp")
                else:
                    pp = pw.tile([128, 512], f32, name="pp", tag="pw")
                nc.tensor.matmul(pp, lhsT=pva8[:, :, ot * 128:(ot + 1) * 128],
                                 rhs=u8[:, :, ns], start=True, stop=True,
                                 perf_mode=DR)
                ot_t = work.tile([128, 512], f32, name="ot_t", tag="ot_t")
                nc.vector.scalar_tensor_tensor(out=ot_t, in0=pp,
                                               scalar=1.0 / 64.0,
                                               in1=xqb[:, ot, ns], op0=MULT,
                                               op1=ADD)
                nc.sync.dma_start(out=out_d[:, NQ * ot + 512 * nch:
                                            NQ * ot + 512 * (nch + 1)],
                                  in_=ot_t)

    nc.compile()
    return nc


def _get_nc():
    key = "nc"
    if key not in _CACHE:
        _CACHE[key] = _build_nc()
    return _CACHE[key]


def _host_inputs(x, gamma, beta, Wq, bq, Wk, bk, Wv, bv, Wp, bp):
    import ml_dtypes
    f8 = ml_dtypes.float8_e4m3
    bf = ml_dtypes.bfloat16

    x = np.asarray(x, np.float32)
    xf = np.ascontiguousarray(x.reshape(2, C, N))
    gamma = np.asarray(gamma, np.float32)
    beta = np.asarray(beta, np.float32)
    Wq, Wk, Wv, Wp = [np.asarray(W, np.float32) for W in (Wq, Wk, Wv, Wp)]
    bq, bv, bp = [np.asarray(v, np.float32) for v in (bq, bv, bp)]

    # host-fused matrices: G = Wk^T Wq, Pv = Wp @ Wv
    G = Wk.T @ Wq                                             # [c, c']
    Pv = Wp @ Wv                                              # [o, c]
    # gt[p, i, c] = G[c, i*128+p]  (contraction over c' = i*128+p)
    gt = np.ascontiguousarray(
        G.T.reshape(2, 128, C).transpose(1, 0, 2).reshape(128, 2 * C)
    ).astype(bf)
    # pvt[p, i, o] = Pv[o, i*128+p]; the r-matmul's 1/64 stationary makes
    # u8 = 64*u/r (fp8 normal range), repaid by the epilogue's /64.
    pvt = np.ascontiguousarray(
        Pv.T.reshape(2, 128, C).transpose(1, 0, 2).reshape(128, 2 * C)
    ).astype(bf)
    # wall[p, i, (wv|wp), o] = W[o, i*128+p]
    wall = np.stack([Wv.T, Wp.T], axis=1)                     # [c, 2, o]
    wall = wall.reshape(2, 128, 2, C).transpose(1, 0, 2, 3)
    wall = np.ascontiguousarray(wall.reshape(128, 2 * 2 * C)).astype(bf)

    small = np.zeros((128, 26), np.float32)
    small[:, 0:2] = gamma.reshape(2, 128).T
    small[:, 2:4] = beta.reshape(2, 128).T
    small[:, 4:6] = bv.reshape(2, 128).T
    small[:, 6:8] = (Wk.T @ bq).reshape(2, 128).T
    small[:, 8:10] = bp.reshape(2, 128).T
    cids = np.arange(C)
    gm = np.zeros((C, GROUPS), np.float32)
    gm[cids, cids // GSIZE] = 1.0                             # [c, g]
    gmT = gm.reshape(2, 128, GROUPS).transpose(1, 0, 2)       # [p, i, g]
    small[:, 10:18] = gmT[:, 0, :] / (GSIZE * N)
    small[:, 18:26] = gmT[:, 1, :] / (GSIZE * N)
    gmask8 = np.ascontiguousarray(gm.T)                       # [g, c]
    imask = np.eye(128, dtype=np.float32).astype(bf)

    in_maps = []
    for core in range(8):
        b, j = divmod(core, 4)
        xrot = np.roll(xf[b], -j * NQ, axis=1)                # [C, N]
        x8 = xrot.astype(f8)
        xb8 = np.ascontiguousarray(
            x8.reshape(2, 128, N).transpose(1, 0, 2).reshape(128, 2 * N))
        dxq = (xrot[:, :NQ] - x8[:, :NQ].astype(np.float32))
        dxq = np.ascontiguousarray(
            dxq.reshape(2, 128, NQ).transpose(1, 0, 2).reshape(128, 2 * NQ)
        ).astype(bf)
        # xt8: position-major with ones cols: per n: [c0..c127, 1, c128.., 1]
        xt = x8.astype(np.float32).T                          # [n, c] quantized
        arr = np.zeros((N, 272), np.float32)
        arr[:, 0:128] = xt[:, 0:128]
        arr[:, 128] = 1.0
        arr[:, 136:264] = xt[:, 128:256]
        arr[:, 264] = 1.0
        xt8 = np.ascontiguousarray(
            arr.reshape(32, 128, 272).transpose(1, 0, 2).reshape(128, 32 * 272)
        ).astype(f8)
        in_maps.append({
            "xt8": xt8, "xb8": xb8, "gt": gt, "pvt": pvt, "wall": wall,
            "small": small, "gmask8": gmask8, "imask": imask,
            "dxq": dxq,
        })
    return in_maps


def kernel(x, gamma, beta, Wq, bq, Wk, bk, Wv, bv, Wp, bp):
    from concourse.bass_utils import run_bass_kernel_spmd
    global LAST_RESULTS

    orig_shape = np.asarray(x).shape
    in_maps = _host_inputs(x, gamma, beta, Wq, bq, Wk, bk, Wv, bv, Wp, bp)
    nc = _get_nc()

    trace = os.environ.get("BASSK_TRACE", "0") == "1"
    res = run_bass_kernel_spmd(nc, in_maps, core_ids=list(range(8)), trace=trace)
    LAST_RESULTS = res

    out = np.empty((2, C, N), np.float32)
    for core in range(8):
        b, j = divmod(core, 4)
        o = res.results[core]["out"]                          # [128, 2*NQ]
        o = o.reshape(128, 2, NQ).transpose(1, 0, 2).reshape(C, NQ)
        out[b][:, j * NQ:(j + 1) * NQ] = o
    return out.reshape(orig_shape)


# revision 44
# speedup vs baseline: 2.1651x; 1.0109x over previous
"""AttentionBlock3D (GroupNorm + single-head self-attention + residual) on 8 TRN2 cores.

Sharding: core = (batch b in {0,1}) x (1024-row slice of the 4096 attention rows).
Each core computes its batch's GroupNorm stats (cheap, on the PE) and
attention + output projection for its own 1024 query rows. No collectives.
The host ROTATES each core's x copy so that its query rows are always
columns 0..1024 (attention is permutation-invariant over keys).

fp8 DoubleRow pipeline with K and V eliminated:
  - x ships twice in fp8 e4m3: xb8 [128, 2, N] (channel-major) and
    xt8 [128, 32, 258] (position-major with built-in ones columns).
  - GroupNorm stats on the PE: x^T x accumulated per channel half; the ones
    column gives Sum x, the diagonal (mask reduce) gives Sum x^2.
  - S = hn^T G hn_q with G = Wk^T Wq (host): q' = (G.A-folded)^T x_q scaled
    by A, S = x^T q' via DoubleRow with xb8 stationary — no K tensor.
    Per-key bias delta[m] = (A o (G B + Wk^T bq))^T x[:,m] enters S as one
    extra DoubleRow matmul with constant-column rhs (coef broadcast).
  - u = x^T E via DoubleRow with xt8 stationary — no V tensor; the output
    projection fuses Pv = Wp @ Wv (host) with A folded on device:
    out = x_q + bpe + (Pv.A)^T u / r,  bpe = bp + Wp cv, cv = Wv B + bv.
  - E = exp(S/16 - 2) on ACT in [128,1024] tiles (e^-2 cancels against r).
    ACT exp is the bottleneck engine (~4.2M elements/core).
"""

import os
import numpy as np
from contextlib import ExitStack

C = 256          # channels
N = 4096         # spatial positions (16*16*16)
NQ = 1024        # query rows per core
GROUPS = 8
GSIZE = C // GROUPS
EPS = 1e-5

_CACHE = {}
LAST_RESULTS = None  # test harness can inspect trace results


def _build_nc():
    import concourse.bacc as bacc
    import concourse.tile as tile
    from concourse import mybir

    f32 = mybir.dt.float32
    f32r = mybir.dt.float32r
    bf16 = mybir.dt.bfloat16
    f8 = mybir.dt.float8e4
    AF = mybir.ActivationFunctionType
    DR = mybir.MatmulPerfMode.DoubleRow
    ADD = mybir.AluOpType.add
    MULT = mybir.AluOpType.mult

    nc = bacc.Bacc("TRN2", target_bir_lowering=False, debug=False,
                   enable_asserts=False)

    # ---- DRAM I/O (per-core) ----
    xt8_d = nc.dram_tensor("xt8", [128, 32 * 272], f8, kind="ExternalInput").ap()
    xb8_d = nc.dram_tensor("xb8", [128, 2 * N], f8, kind="ExternalInput").ap()
    gt_d = nc.dram_tensor("gt", [128, 2 * C], bf16, kind="ExternalInput").ap()
    pvt_d = nc.dram_tensor("pvt", [128, 2 * C], bf16, kind="ExternalInput").ap()
    wall_d = nc.dram_tensor("wall", [128, 2 * 2 * C], bf16, kind="ExternalInput").ap()
    small_d = nc.dram_tensor("small", [128, 26], f32, kind="ExternalInput").ap()
    gmask8_d = nc.dram_tensor("gmask8", [GROUPS, C], f32, kind="ExternalInput").ap()
    imask_d = nc.dram_tensor("imask", [128, 128], bf16, kind="ExternalInput").ap()
    dxq_d = nc.dram_tensor("dxq", [128, 2 * NQ], bf16, kind="ExternalInput").ap()
    out_d = nc.dram_tensor("out", [128, 2 * NQ], f32, kind="ExternalOutput").ap()

    with tile.TileContext(nc) as tc, ExitStack() as ctx:
        big = ctx.enter_context(tc.tile_pool(name="big", bufs=1))
        consts = ctx.enter_context(tc.tile_pool(name="consts", bufs=1))
        work = ctx.enter_context(tc.tile_pool(name="work", bufs=4))
        epool = ctx.enter_context(tc.tile_pool(name="epool", bufs=5))
        # PSUM banks: psp 2x2 + pacc 2 + pr 1 + pw 1 = 8
        psp = ctx.enter_context(tc.tile_pool(name="psp", bufs=2, space="PSUM"))
        pacc = ctx.enter_context(tc.tile_pool(name="pacc", bufs=2, space="PSUM"))
        pr = ctx.enter_context(tc.tile_pool(name="pr", bufs=1, space="PSUM"))
        pw = ctx.enter_context(tc.tile_pool(name="pw", bufs=1, space="PSUM"))

        # ---- constants (before the big loads) ----
        # r-matmul stationary = 1/64 so the reciprocal yields 64/r; the /64
        # is repaid in the epilogue stt. Keeps u8 = 64*u/r in fp8's sweet spot.
        ones8 = consts.tile([128, 2, 128], f8)
        nc.vector.memset(ones8, 1.0 / 64.0)
        nbias = consts.tile([128, 1], f32)
        nc.vector.memset(nbias, -3.5)
        # eps8 = Sqrt(EPS^2) on ACT: forces the Sqrt act-table load at t~0
        eps_sq = consts.tile([GROUPS, 1], f32)
        nc.vector.memset(eps_sq, EPS * EPS)
        eps8 = consts.tile([GROUPS, 1], f32)
        nc.scalar.activation(out=eps8, in_=eps_sq, func=AF.Sqrt, scale=1.0)

        # ---- load xt8 (stats + u path) chunked; stats matmuls interleave ----
        xt8 = big.tile([128, 32, 272], f8)
        for ch in range(4):
            nc.sync.dma_start(out=xt8[:, 8 * ch:8 * ch + 8, :],
                              in_=xt8_d[:, 8 * 272 * ch:8 * 272 * (ch + 1)])
        # xx[h] accumulates x^T x for channel half h ([128,129]: 128 cols of
        # the x^T x block + ones column giving Sum x). Uses the idle sp slots.
        xx = []
        for h in range(2):
            t = psp.tile([128, 129], f32, name=f"xx{h}", tag="sp")
            xx.append(t)
        for s in range(16):
            for h in range(2):
                nc.tensor.matmul(
                    xx[h],
                    lhsT=xt8[:, 2 * s:2 * s + 2, 136 * h:136 * h + 128],
                    rhs=xt8[:, 2 * s:2 * s + 2, 136 * h:136 * h + 129],
                    start=(s == 0), stop=(s == 15), perf_mode=DR)

        # ---- smalls + G^T + xb8 query cols (early, for q') ----
        small_sb = consts.tile([128, 26], f32)
        nc.sync.dma_start(out=small_sb, in_=small_d)
        imask = consts.tile([128, 128], bf16)
        nc.sync.dma_start(out=imask, in_=imask_d)
        gmask8 = consts.tile([GROUPS, C], f32)
        nc.sync.dma_start(out=gmask8, in_=gmask8_d)
        gamma2 = small_sb[:, 0:2]
        beta2 = small_sb[:, 2:4]
        bv2 = small_sb[:, 4:6]
        wtld2 = small_sb[:, 6:8]          # Wk^T bq (host)
        bp2 = small_sb[:, 8:10]
        gmaskT = [small_sb[:, 10 + 8 * i:18 + 8 * i] for i in range(2)]  # pre-scaled 1/(32N)
        gt = consts.tile([128, 2, C], bf16)
        nc.sync.dma_start(out=gt, in_=gt_d)
        xb8 = big.tile([128, 2, N], f8)
        for i in range(2):
            nc.sync.dma_start(out=xb8[:, i, 0:NQ], in_=xb8_d[:, N * i:N * i + NQ])

        # ---- group stats -> per-channel A (f32) and B (bf16) ----
        # Sum x^2 = diag(x^T x): mask out the diagonal, then column-sum it
        # back to [128,1] with a N=1 matmul (diag matrix -> col sums = diag).
        ones_col = consts.tile([128, 2], f32)
        nc.vector.memset(ones_col, 1.0)
        ones_colr = consts.tile([128, 2], f32r)
        nc.vector.tensor_copy(ones_colr, ones_col)
        stile = work.tile([128, 2, 2], f32, name="stile")  # [:, h, (sx, sxx)]
        for h in range(2):
            nc.vector.tensor_copy(stile[:, h, 0:1], xx[h][:, 128:129])
            scr = work.tile([128, 128], f32r, name="scr", tag="scr", bufs=2)
            nc.vector.tensor_mul(scr, xx[h][:, 0:128], imask)
            sxpool = pw if h == 0 else pr
            sxp = sxpool.tile([128, 2], f32, name="sxp", tag="pw" if h == 0 else "r")
            nc.tensor.matmul(sxp, lhsT=scr, rhs=ones_colr, start=True, stop=True)
            nc.vector.tensor_copy(stile[:, h, 1:2], sxp[:, 0:1])
        gp = pacc.tile([GROUPS, 2], f32, tag="u")
        for h in range(2):
            nc.tensor.matmul(gp, lhsT=gmaskT[h], rhs=stile[:, h, :],
                             start=(h == 0), stop=(h == 1))
        # gmaskT is host-scaled by 1/(32N), so gp = (mean, E[x^2]) directly.
        gsb = work.tile([GROUPS, 2], f32, name="gsb")
        nc.vector.tensor_copy(gsb, gp)
        negvar = work.tile([GROUPS, 1], f32, name="negvar")
        nc.vector.scalar_tensor_tensor(out=negvar, in0=gsb[:, 0:1],
                                       scalar=gsb[:, 0:1], in1=gsb[:, 1:2],
                                       op0=MULT, op1=mybir.AluOpType.subtract)
        gsd = work.tile([GROUPS, 1], f32, name="gsd")
        nc.scalar.activation(out=gsd, in_=negvar, func=AF.Sqrt, bias=eps8,
                             scale=-1.0)
        # preload the Exp act table now (the Sqrt above was ACT's last
        # non-Exp op; loading here keeps the main loop table-stable)
        dummye = consts.tile([1, 1], f32)
        nc.scalar.activation(out=dummye, in_=gsd[0:1, :], func=AF.Exp,
                             scale=1.0, bias=nbias[0:1, :])
        nc.vector.reciprocal(out=gsb[:, 1:2], in_=gsd)

        # B2 holds -B = mean*A - beta (sign fixed up at the consumers)
        A2 = consts.tile([128, 2], f32)
        B2 = consts.tile([128, 2], bf16)
        B2f = work.tile([128, 2], f32, name="B2f")
        for i in range(2):
            gbp = pw.tile([128, 2], f32, name="gbp", tag="pw")
            nc.tensor.matmul(gbp, lhsT=gmask8[:, 128 * i:128 * (i + 1)],
                             rhs=gsb, start=True, stop=True)
            nc.vector.tensor_mul(A2[:, i:i + 1], gamma2[:, i:i + 1], gbp[:, 1:2])
            nc.vector.scalar_tensor_tensor(out=B2f[:, i:i + 1], in0=gbp[:, 0:1],
                                           scalar=A2[:, i:i + 1],
                                           in1=beta2[:, i:i + 1], op0=MULT,
                                           op1=mybir.AluOpType.subtract)
        nc.vector.tensor_copy(B2, B2f)

        # ---- delta coefficient: coef = A o (G B + Wk^T bq) ----
        # Folded into the q' drain: q8 = A*qp + coef makes the single
        # S matmul compute S + delta[m] directly (delta const over n).
        gbv = pacc.tile([128, 2], f32, name="gbv", tag="u")
        for ch in range(2):
            for i in range(2):
                nc.tensor.matmul(gbv[:, ch:ch + 1],
                                 lhsT=gt[:, i, ch * 128:(ch + 1) * 128],
                                 rhs=B2[:, i:i + 1], start=(i == 0), stop=(i == 1))
        coef = work.tile([128, 2], f32, name="coef")
        nc.vector.tensor_sub(coef, wtld2, gbv)     # gbv = -G B
        nc.vector.tensor_mul(coef, coef, A2)

        # ---- fold A into G^T -> fp8 (contraction-side fold for q') ----
        ga8 = consts.tile([128, 2, C], f8)
        for i in range(2):
            nc.vector.tensor_scalar_mul(ga8[:, i, :], gt[:, i, :], A2[:, i:i + 1])

        # ---- q'[c, n] = A[c] * sum_c' (G[c,c'] A[c']) x_q[c', n] + coef[c] ----
        q8 = big.tile([128, 2, NQ], f8)

        def emit_q(qc, chans=(0, 1)):
            for ch in chans:
                ns = slice(qc * 512, (qc + 1) * 512)
                if qc == 0:
                    qp = psp.tile([128, 512], f32, name="qp", tag="sp")
                else:
                    qp = pw.tile([128, 512], f32, name="qp", tag="pw")
                nc.tensor.matmul(qp, lhsT=ga8[:, :, ch * 128:(ch + 1) * 128],
                                 rhs=xb8[:, :, ns], start=True, stop=True,
                                 perf_mode=DR)
                if qc == 0 and ch == 1:
                    # ACT is idle pre-stream; halves the head's drain chain
                    nc.scalar.activation(out=q8[:, ch, ns], in_=qp,
                                         func=AF.Identity,
                                         scale=A2[:, ch:ch + 1],
                                         bias=coef[:, ch:ch + 1])
                else:
                    nc.vector.tensor_scalar(out=q8[:, ch, ns], in0=qp,
                                            scalar1=A2[:, ch:ch + 1],
                                            scalar2=coef[:, ch:ch + 1],
                                            op0=MULT, op1=ADD)

        emit_q(0)
        nch_tail = [None, None]

        # ---- rest of the loads (xb8 keys, weights for biases, xq last) ----
        for i in range(2):
            nc.sync.dma_start(out=xb8[:, i, NQ:N],
                              in_=xb8_d[:, N * i + NQ:N * (i + 1)])
        dxq = big.tile([128, 2, NQ], bf16)
        nc.sync.dma_start(out=dxq, in_=dxq_d)
        wall = consts.tile([128, 2, 2 * C], bf16)   # [wv, wp]
        nc.sync.dma_start(out=wall, in_=wall_d)
        pvt = consts.tile([128, 2, C], bf16)
        nc.sync.dma_start(out=pvt, in_=pvt_d)

        # ---- fold A into Pv^T -> fp8 (fused Wp @ Wv projection) ----
        pva8 = consts.tile([128, 2, C], f8)
        for i in range(2):
            nc.vector.tensor_scalar_mul(pva8[:, i, :], pvt[:, i, :],
                                        A2[:, i:i + 1])

        # ---- cv = Wv B + bv (bf16); bpe2 = bp + Wp cv; xqb = xq + bpe2 ----
        cvf = work.tile([128, 2], f32, name="cvf")
        for ot in range(2):
            cvpool = pw if ot == 0 else pr
            p = cvpool.tile([128, 1], f32, name="cvp",
                            tag="pw" if ot == 0 else "r")
            for i in range(2):
                nc.tensor.matmul(p, lhsT=wall[:, i, 128 * ot:128 * (ot + 1)],
                                 rhs=B2[:, i:i + 1], start=(i == 0), stop=(i == 1))
            nc.vector.tensor_sub(cvf[:, ot:ot + 1], bv2[:, ot:ot + 1], p)
        cv = consts.tile([128, 2], bf16)
        nc.vector.tensor_copy(cv, cvf)
        bpe2 = work.tile([128, 2], f32, name="bpe2")
        for ot in range(2):
            bppool = pw if ot == 0 else pr
            p2 = bppool.tile([128, 1], f32, name="bpp",
                             tag="pw" if ot == 0 else "r")
            for i in range(2):
                nc.tensor.matmul(p2, lhsT=wall[:, i, C + 128 * ot:C + 128 * (ot + 1)],
                                 rhs=cv[:, i:i + 1], start=(i == 0), stop=(i == 1))
            nc.vector.tensor_scalar_add(bpe2[:, ot:ot + 1], p2, bp2[:, ot:ot + 1])
        # residual x_q reconstructed as fp8(x) + bf16 quantization remainder
        xqb = big.tile([128, 2, NQ], f32)
        for ot in range(2):
            nc.gpsimd.tensor_add(xqb[:, ot, :], xb8[:, ot, 0:NQ], dxq[:, ot, :])
            nc.gpsimd.tensor_scalar_add(xqb[:, ot, :], xqb[:, ot, :],
                                        bpe2[:, ot:ot + 1])

        # ---- attention: S + delta -> exp -> r, u0, u1 -> scale -> project ----
        u8 = big.tile([128, 2, NQ], f8)
        for nch in range(2):
            ns = slice(nch * 512, (nch + 1) * 512)
            rp = pr.tile([128, 512], f32, name="rp", tag="r")
            up = [pacc.tile([128, 512], f32, name=f"up{h}", tag="u")
                  for h in range(2)]
            for s in range(16):
                if nch == 0 and s == 2:
                    emit_q(1, chans=(0,))
                if nch == 0 and s == 4:
                    emit_q(1, chans=(1,))
                if nch == 1 and s == 2 and nch_tail[0] is not None:
                    nch_tail[0]()          # nch0's normalize/project/store
                sp = psp.tile([128, 1024], f32, name="sp", tag="sp")
                for h in range(2):
                    ms = slice((2 * s + h) * 128, (2 * s + h + 1) * 128)
                    hs = slice(512 * h, 512 * (h + 1))
                    nc.tensor.matmul(sp[:, hs], lhsT=xb8[:, :, ms],
                                     rhs=q8[:, :, ns], start=True, stop=True,
                                     perf_mode=DR)
                e = epool.tile([128, 2, 512], f8, name="e", tag="e")
                nc.scalar.activation(out=e, in_=sp, func=AF.Exp,
                                     scale=1.0 / 16.0, bias=nbias)
                nc.tensor.matmul(rp, lhsT=ones8, rhs=e,
                                 start=(s == 0), stop=(s == 15), perf_mode=DR)
                for h in range(2):
                    nc.tensor.matmul(
                        up[h],
                        lhsT=xt8[:, 2 * s:2 * s + 2, 136 * h:136 * h + 128],
                        rhs=e, start=(s == 0), stop=(s == 15), perf_mode=DR)
            rb = work.tile([128, 512], f32, name="rb", tag="rb", bufs=2)
            nc.vector.reciprocal(out=rb, in_=rp)
            for h in range(2):
                nc.vector.tensor_mul(u8[:, h, ns], up[h], rb)

            # fused projection; out = pp/64 + (xq + bpe) in one stt
            for ot in range(2):
                if nch == 1:
                    pp = psp.tile([128, 512], f32, name="pp", tag="sp")
                else:
                    pp = pw.tile([128, 512], f32, name="pp", tag="pw")
                nc.tensor.matmul(pp, lhsT=pva8[:, :, ot * 128:(ot + 1) * 128],
                                 rhs=u8[:, :, ns], start=True, stop=True,
                                 perf_mode=DR)
                ot_t = work.tile([128, 512], f32, name="ot_t", tag="ot_t")
                nc.vector.scalar_tensor_tensor(out=ot_t, in0=pp,
                                               scalar=1.0 / 64.0,
                                               in1=xqb[:, ot, ns], op0=MULT,
                                               op1=ADD)
                nc.sync.dma_start(out=out_d[:, NQ * ot + 512 * nch:
                                            NQ * ot + 512 * (nch + 1)],
                                  in_=ot_t)

    nc.compile()
    return nc


def _get_nc():
    key = "nc"
    if key not in _CACHE:
        _CACHE[key] = _build_nc()
    return _CACHE[key]


def _host_inputs(x, gamma, beta, Wq, bq, Wk, bk, Wv, bv, Wp, bp):
    import ml_dtypes
    f8 = ml_dtypes.float8_e4m3
    bf = ml_dtypes.bfloat16

    x = np.asarray(x, np.float32)
    xf = np.ascontiguousarray(x.reshape(2, C, N))
    gamma = np.asarray(gamma, np.float32)
    beta = np.asarray(beta, np.float32)
    Wq, Wk, Wv, Wp = [np.asarray(W, np.float32) for W in (Wq, Wk, Wv, Wp)]
    bq, bv, bp = [np.asarray(v, np.float32) for v in (bq, bv, bp)]

    # host-fused matrices: G = Wk^T Wq, Pv = Wp @ Wv
    G = Wk.T @ Wq                                             # [c, c']
    Pv = Wp @ Wv                                              # [o, c]
    # gt[p, i, c] = G[c, i*128+p]  (contraction over c' = i*128+p)
    gt = np.ascontiguousarray(
        G.T.reshape(2, 128, C).transpose(1, 0, 2).reshape(128, 2 * C)
    ).astype(bf)
    # pvt[p, i, o] = Pv[o, i*128+p]; the r-matmul's 1/64 stationary makes
    # u8 = 64*u/r (fp8 normal range), repaid by the epilogue's /64.
    pvt = np.ascontiguousarray(
        Pv.T.reshape(2, 128, C).transpose(1, 0, 2).reshape(128, 2 * C)
    ).astype(bf)
    # wall[p, i, (wv|wp), o] = W[o, i*128+p]
    wall = np.stack([Wv.T, Wp.T], axis=1)                     # [c, 2, o]
    wall = wall.reshape(2, 128, 2, C).transpose(1, 0, 2, 3)
    wall = np.ascontiguousarray(wall.reshape(128, 2 * 2 * C)).astype(bf)

    small = np.zeros((128, 26), np.float32)
    small[:, 0:2] = gamma.reshape(2, 128).T
    small[:, 2:4] = beta.reshape(2, 128).T
    small[:, 4:6] = bv.reshape(2, 128).T
    small[:, 6:8] = (Wk.T @ bq).reshape(2, 128).T
    small[:, 8:10] = bp.reshape(2, 128).T
    cids = np.arange(C)
    gm = np.zeros((C, GROUPS), np.float32)
    gm[cids, cids // GSIZE] = 1.0                             # [c, g]
    gmT = gm.reshape(2, 128, GROUPS).transpose(1, 0, 2)       # [p, i, g]
    small[:, 10:18] = gmT[:, 0, :] / (GSIZE * N)
    small[:, 18:26] = gmT[:, 1, :] / (GSIZE * N)
    gmask8 = np.ascontiguousarray(gm.T)                       # [g, c]
    imask = np.eye(128, dtype=np.float32).astype(bf)

    in_maps = []
    for core in range(8):
        b, j = divmod(core, 4)
        xrot = np.roll(xf[b], -j * NQ, axis=1)                # [C, N]
        x8 = xrot.astype(f8)
        xb8 = np.ascontiguousarray(
            x8.reshape(2, 128, N).transpose(1, 0, 2).reshape(128, 2 * N))
        dxq = (xrot[:, :NQ] - x8[:, :NQ].astype(np.float32))
        dxq = np.ascontiguousarray(
            dxq.reshape(2, 128, NQ).transpose(1, 0, 2).reshape(128, 2 * NQ)
        ).astype(bf)
        # xt8: position-major with ones cols: per n: [c0..c127, 1, c128.., 1]
        xt = x8.astype(np.float32).T                          # [n, c] quantized
        arr = np.zeros((N, 272), np.float32)
        arr[:, 0:128] = xt[:, 0:128]
        arr[:, 128] = 1.0
        arr[:, 136:264] = xt[:, 128:256]
        arr[:, 264] = 1.0
        xt8 = np.ascontiguousarray(
            arr.reshape(32, 128, 272).transpose(1, 0, 2).reshape(128, 32 * 272)
        ).astype(f8)
        in_maps.append({
            "xt8": xt8, "xb8": xb8, "gt": gt, "pvt": pvt, "wall": wall,
            "small": small, "gmask8": gmask8, "imask": imask,
            "dxq": dxq,
        })
    return in_maps


def kernel(x, gamma, beta, Wq, bq, Wk, bk, Wv, bv, Wp, bp):
    from concourse.bass_utils import run_bass_kernel_spmd
    global LAST_RESULTS

    orig_shape = np.asarray(x).shape
    in_maps = _host_inputs(x, gamma, beta, Wq, bq, Wk, bk, Wv, bv, Wp, bp)
    nc = _get_nc()

    trace = os.environ.get("BASSK_TRACE", "0") == "1"
    res = run_bass_kernel_spmd(nc, in_maps, core_ids=list(range(8)), trace=trace)
    LAST_RESULTS = res

    out = np.empty((2, C, N), np.float32)
    for core in range(8):
        b, j = divmod(core, 4)
        o = res.results[core]["out"]                          # [128, 2*NQ]
        o = o.reshape(128, 2, NQ).transpose(1, 0, 2).reshape(C, NQ)
        out[b][:, j * NQ:(j + 1) * NQ] = o
    return out.reshape(orig_shape)


# revision 46
# speedup vs baseline: 2.1812x; 1.0074x over previous
"""AttentionBlock3D (GroupNorm + single-head self-attention + residual) on 8 TRN2 cores.

Sharding: core = (batch b in {0,1}) x (1024-row slice of the 4096 attention rows).
Each core computes its batch's GroupNorm stats (cheap, on the PE) and
attention + output projection for its own 1024 query rows. No collectives.
The host ROTATES each core's x copy so that its query rows are always
columns 0..1024 (attention is permutation-invariant over keys).

fp8 DoubleRow pipeline with K and V eliminated:
  - x ships twice in fp8 e4m3: xb8 [128, 2, N] (channel-major) and
    xt8 [128, 32, 258] (position-major with built-in ones columns).
  - GroupNorm stats on the PE: x^T x accumulated per channel half; the ones
    column gives Sum x, the diagonal (mask reduce) gives Sum x^2.
  - S = hn^T G hn_q with G = Wk^T Wq (host): q' = (G.A-folded)^T x_q scaled
    by A, S = x^T q' via DoubleRow with xb8 stationary — no K tensor.
    Per-key bias delta[m] = (A o (G B + Wk^T bq))^T x[:,m] enters S as one
    extra DoubleRow matmul with constant-column rhs (coef broadcast).
  - u = x^T E via DoubleRow with xt8 stationary — no V tensor; the output
    projection fuses Pv = Wp @ Wv (host) with A folded on device:
    out = x_q + bpe + (Pv.A)^T u / r,  bpe = bp + Wp cv, cv = Wv B + bv.
  - E = exp(S/16 - 2) on ACT in [128,1024] tiles (e^-2 cancels against r).
    ACT exp is the bottleneck engine (~4.2M elements/core).
"""

import os
import numpy as np
from contextlib import ExitStack

C = 256          # channels
N = 4096         # spatial positions (16*16*16)
NQ = 1024        # query rows per core
GROUPS = 8
GSIZE = C // GROUPS
EPS = 1e-5

_CACHE = {}
LAST_RESULTS = None  # test harness can inspect trace results


def _build_nc():
    import concourse.bacc as bacc
    import concourse.tile as tile
    from concourse import mybir

    f32 = mybir.dt.float32
    f32r = mybir.dt.float32r
    bf16 = mybir.dt.bfloat16
    f8 = mybir.dt.float8e4
    AF = mybir.ActivationFunctionType
    DR = mybir.MatmulPerfMode.DoubleRow
    ADD = mybir.AluOpType.add
    MULT = mybir.AluOpType.mult

    nc = bacc.Bacc("TRN2", target_bir_lowering=False, debug=False,
                   enable_asserts=False)

    # ---- DRAM I/O (per-core) ----
    xt8_d = nc.dram_tensor("xt8", [128, 32 * 272], f8, kind="ExternalInput").ap()
    xb8_d = nc.dram_tensor("xb8", [128, 2 * N], f8, kind="ExternalInput").ap()
    gt_d = nc.dram_tensor("gt", [128, 2 * C], bf16, kind="ExternalInput").ap()
    pvt_d = nc.dram_tensor("pvt", [128, 2 * C], bf16, kind="ExternalInput").ap()
    wall_d = nc.dram_tensor("wall", [128, 2 * 2 * C], bf16, kind="ExternalInput").ap()
    small_d = nc.dram_tensor("small", [128, 26], f32, kind="ExternalInput").ap()
    gmask8_d = nc.dram_tensor("gmask8", [GROUPS, C], f32, kind="ExternalInput").ap()
    imask_d = nc.dram_tensor("imask", [128, 128], bf16, kind="ExternalInput").ap()
    dxq_d = nc.dram_tensor("dxq", [128, 2 * NQ], bf16, kind="ExternalInput").ap()
    out_d = nc.dram_tensor("out", [128, 2 * NQ], bf16, kind="ExternalOutput").ap()

    with tile.TileContext(nc) as tc, ExitStack() as ctx:
        big = ctx.enter_context(tc.tile_pool(name="big", bufs=1))
        consts = ctx.enter_context(tc.tile_pool(name="consts", bufs=1))
        work = ctx.enter_context(tc.tile_pool(name="work", bufs=4))
        epool = ctx.enter_context(tc.tile_pool(name="epool", bufs=5))
        # PSUM banks: psp 2x2 + pacc 2 + pr 1 + pw 1 = 8
        psp = ctx.enter_context(tc.tile_pool(name="psp", bufs=2, space="PSUM"))
        pacc = ctx.enter_context(tc.tile_pool(name="pacc", bufs=2, space="PSUM"))
        pr = ctx.enter_context(tc.tile_pool(name="pr", bufs=1, space="PSUM"))
        pw = ctx.enter_context(tc.tile_pool(name="pw", bufs=1, space="PSUM"))

        # ---- constants (before the big loads) ----
        # r-matmul stationary = 1/64 so the reciprocal yields 64/r; the /64
        # is repaid in the epilogue stt. Keeps u8 = 64*u/r in fp8's sweet spot.
        ones8 = consts.tile([128, 2, 128], f8)
        nc.vector.memset(ones8, 1.0 / 64.0)
        nbias = consts.tile([128, 1], f32)
        nc.vector.memset(nbias, -3.5)
        # eps8 = Sqrt(EPS^2) on ACT: forces the Sqrt act-table load at t~0
        eps_sq = consts.tile([GROUPS, 1], f32)
        nc.vector.memset(eps_sq, EPS * EPS)
        eps8 = consts.tile([GROUPS, 1], f32)
        nc.scalar.activation(out=eps8, in_=eps_sq, func=AF.Sqrt, scale=1.0)

        # ---- load xt8 (stats + u path) chunked; stats matmuls interleave ----
        xt8 = big.tile([128, 32, 272], f8)
        for ch in range(4):
            nc.sync.dma_start(out=xt8[:, 8 * ch:8 * ch + 8, :],
                              in_=xt8_d[:, 8 * 272 * ch:8 * 272 * (ch + 1)])
        # xx[h] accumulates x^T x for channel half h ([128,129]: 128 cols of
        # the x^T x block + ones column giving Sum x). Uses the idle sp slots.
        xx = []
        for h in range(2):
            t = psp.tile([128, 129], f32, name=f"xx{h}", tag="sp")
            xx.append(t)
        for s in range(16):
            for h in range(2):
                nc.tensor.matmul(
                    xx[h],
                    lhsT=xt8[:, 2 * s:2 * s + 2, 136 * h:136 * h + 128],
                    rhs=xt8[:, 2 * s:2 * s + 2, 136 * h:136 * h + 129],
                    start=(s == 0), stop=(s == 15), perf_mode=DR)

        # ---- smalls + G^T + xb8 query cols (early, for q') ----
        small_sb = consts.tile([128, 26], f32)
        nc.sync.dma_start(out=small_sb, in_=small_d)
        imask = consts.tile([128, 128], bf16)
        nc.sync.dma_start(out=imask, in_=imask_d)
        gmask8 = consts.tile([GROUPS, C], f32)
        nc.sync.dma_start(out=gmask8, in_=gmask8_d)
        gamma2 = small_sb[:, 0:2]
        beta2 = small_sb[:, 2:4]
        bv2 = small_sb[:, 4:6]
        wtld2 = small_sb[:, 6:8]          # Wk^T bq (host)
        bp2 = small_sb[:, 8:10]
        gmaskT = [small_sb[:, 10 + 8 * i:18 + 8 * i] for i in range(2)]  # pre-scaled 1/(32N)
        gt = consts.tile([128, 2, C], bf16)
        nc.sync.dma_start(out=gt, in_=gt_d)
        xb8 = big.tile([128, 2, N], f8)
        for i in range(2):
            nc.sync.dma_start(out=xb8[:, i, 0:NQ], in_=xb8_d[:, N * i:N * i + NQ])

        # ---- group stats -> per-channel A (f32) and B (bf16) ----
        # Sum x^2 = diag(x^T x): mask out the diagonal, then column-sum it
        # back to [128,1] with a N=1 matmul (diag matrix -> col sums = diag).
        ones_col = consts.tile([128, 2], f32)
        nc.vector.memset(ones_col, 1.0)
        ones_colr = consts.tile([128, 2], f32r)
        nc.vector.tensor_copy(ones_colr, ones_col)
        stile = work.tile([128, 2, 2], f32, name="stile")  # [:, h, (sx, sxx)]
        for h in range(2):
            nc.vector.tensor_copy(stile[:, h, 0:1], xx[h][:, 128:129])
            scr = work.tile([128, 128], f32r, name="scr", tag="scr", bufs=2)
            nc.vector.tensor_mul(scr, xx[h][:, 0:128], imask)
            sxpool = pw if h == 0 else pr
            sxp = sxpool.tile([128, 2], f32, name="sxp", tag="pw" if h == 0 else "r")
            nc.tensor.matmul(sxp, lhsT=scr, rhs=ones_colr, start=True, stop=True)
            nc.vector.tensor_copy(stile[:, h, 1:2], sxp[:, 0:1])
        gp = pacc.tile([GROUPS, 2], f32, tag="u")
        for h in range(2):
            nc.tensor.matmul(gp, lhsT=gmaskT[h], rhs=stile[:, h, :],
                             start=(h == 0), stop=(h == 1))
        # gmaskT is host-scaled by 1/(32N), so gp = (mean, E[x^2]) directly.
        gsb = work.tile([GROUPS, 2], f32, name="gsb")
        nc.vector.tensor_copy(gsb, gp)
        negvar = work.tile([GROUPS, 1], f32, name="negvar")
        nc.vector.scalar_tensor_tensor(out=negvar, in0=gsb[:, 0:1],
                                       scalar=gsb[:, 0:1], in1=gsb[:, 1:2],
                                       op0=MULT, op1=mybir.AluOpType.subtract)
        gsd = work.tile([GROUPS, 1], f32, name="gsd")
        nc.scalar.activation(out=gsd, in_=negvar, func=AF.Sqrt, bias=eps8,
                             scale=-1.0)
        # preload the Exp act table now (the Sqrt above was ACT's last
        # non-Exp op; loading here keeps the main loop table-stable)
        dummye = consts.tile([1, 1], f32)
        nc.scalar.activation(out=dummye, in_=gsd[0:1, :], func=AF.Exp,
                             scale=1.0, bias=nbias[0:1, :])
        nc.vector.reciprocal(out=gsb[:, 1:2], in_=gsd)

        # B2 holds -B = mean*A - beta (sign fixed up at the consumers)
        A2 = consts.tile([128, 2], f32)
        B2 = consts.tile([128, 2], bf16)
        B2f = work.tile([128, 2], f32, name="B2f")
        for i in range(2):
            gbp = pw.tile([128, 2], f32, name="gbp", tag="pw")
            nc.tensor.matmul(gbp, lhsT=gmask8[:, 128 * i:128 * (i + 1)],
                             rhs=gsb, start=True, stop=True)
            nc.vector.tensor_mul(A2[:, i:i + 1], gamma2[:, i:i + 1], gbp[:, 1:2])
            nc.vector.scalar_tensor_tensor(out=B2f[:, i:i + 1], in0=gbp[:, 0:1],
                                           scalar=A2[:, i:i + 1],
                                           in1=beta2[:, i:i + 1], op0=MULT,
                                           op1=mybir.AluOpType.subtract)
        nc.vector.tensor_copy(B2, B2f)

        # ---- delta coefficient: coef = A o (G B + Wk^T bq) ----
        # Folded into the q' drain: q8 = A*qp + coef makes the single
        # S matmul compute S + delta[m] directly (delta const over n).
        gbv = pacc.tile([128, 2], f32, name="gbv", tag="u")
        for ch in range(2):
            for i in range(2):
                nc.tensor.matmul(gbv[:, ch:ch + 1],
                                 lhsT=gt[:, i, ch * 128:(ch + 1) * 128],
                                 rhs=B2[:, i:i + 1], start=(i == 0), stop=(i == 1))
        coef = work.tile([128, 2], f32, name="coef")
        nc.vector.tensor_sub(coef, wtld2, gbv)     # gbv = -G B
        nc.vector.tensor_mul(coef, coef, A2)

        # ---- fold A into G^T -> fp8 (contraction-side fold for q') ----
        ga8 = consts.tile([128, 2, C], f8)
        for i in range(2):
            nc.vector.tensor_scalar_mul(ga8[:, i, :], gt[:, i, :], A2[:, i:i + 1])

        # ---- q'[c, n] = A[c] * sum_c' (G[c,c'] A[c']) x_q[c', n] + coef[c] ----
        q8 = big.tile([128, 2, NQ], f8)

        def emit_q(qc, chans=(0, 1)):
            for ch in chans:
                ns = slice(qc * 512, (qc + 1) * 512)
                if qc == 0:
                    qp = psp.tile([128, 512], f32, name="qp", tag="sp")
                else:
                    qp = pw.tile([128, 512], f32, name="qp", tag="pw")
                nc.tensor.matmul(qp, lhsT=ga8[:, :, ch * 128:(ch + 1) * 128],
                                 rhs=xb8[:, :, ns], start=True, stop=True,
                                 perf_mode=DR)
                if qc == 0 and ch == 1:
                    # ACT is idle pre-stream; halves the head's drain chain
                    nc.scalar.activation(out=q8[:, ch, ns], in_=qp,
                                         func=AF.Identity,
                                         scale=A2[:, ch:ch + 1],
                                         bias=coef[:, ch:ch + 1])
                else:
                    nc.vector.tensor_scalar(out=q8[:, ch, ns], in0=qp,
                                            scalar1=A2[:, ch:ch + 1],
                                            scalar2=coef[:, ch:ch + 1],
                                            op0=MULT, op1=ADD)

        emit_q(0)
        nch_tail = [None, None]

        # ---- rest of the loads (xb8 keys, weights for biases, xq last) ----
        for i in range(2):
            nc.sync.dma_start(out=xb8[:, i, NQ:N],
                              in_=xb8_d[:, N * i + NQ:N * (i + 1)])
        dxq = big.tile([128, 2, NQ], bf16)
        nc.sync.dma_start(out=dxq, in_=dxq_d)
        wall = consts.tile([128, 2, 2 * C], bf16)   # [wv, wp]
        nc.sync.dma_start(out=wall, in_=wall_d)
        pvt = consts.tile([128, 2, C], bf16)
        nc.sync.dma_start(out=pvt, in_=pvt_d)

        # ---- fold A into Pv^T -> fp8 (fused Wp @ Wv projection) ----
        pva8 = consts.tile([128, 2, C], f8)
        for i in range(2):
            nc.vector.tensor_scalar_mul(pva8[:, i, :], pvt[:, i, :],
                                        A2[:, i:i + 1])

        # ---- cv = Wv B + bv (bf16); bpe2 = bp + Wp cv; xqb = xq + bpe2 ----
        cvf = work.tile([128, 2], f32, name="cvf")
        for ot in range(2):
            cvpool = pw if ot == 0 else pr
            p = cvpool.tile([128, 1], f32, name="cvp",
                            tag="pw" if ot == 0 else "r")
            for i in range(2):
                nc.tensor.matmul(p, lhsT=wall[:, i, 128 * ot:128 * (ot + 1)],
                                 rhs=B2[:, i:i + 1], start=(i == 0), stop=(i == 1))
            nc.vector.tensor_sub(cvf[:, ot:ot + 1], bv2[:, ot:ot + 1], p)
        cv = consts.tile([128, 2], bf16)
        nc.vector.tensor_copy(cv, cvf)
        bpe2 = work.tile([128, 2], f32, name="bpe2")
        for ot in range(2):
            bppool = pw if ot == 0 else pr
            p2 = bppool.tile([128, 1], f32, name="bpp",
                             tag="pw" if ot == 0 else "r")
            for i in range(2):
                nc.tensor.matmul(p2, lhsT=wall[:, i, C + 128 * ot:C + 128 * (ot + 1)],
                                 rhs=cv[:, i:i + 1], start=(i == 0), stop=(i == 1))
            nc.vector.tensor_scalar_add(bpe2[:, ot:ot + 1], p2, bp2[:, ot:ot + 1])
        # residual x_q reconstructed as fp8(x) + bf16 quantization remainder
        xqb = big.tile([128, 2, NQ], f32)
        for ot in range(2):
            nc.gpsimd.tensor_add(xqb[:, ot, :], xb8[:, ot, 0:NQ], dxq[:, ot, :])
            nc.gpsimd.tensor_scalar_add(xqb[:, ot, :], xqb[:, ot, :],
                                        bpe2[:, ot:ot + 1])

        # ---- attention: S + delta -> exp -> r, u0, u1 -> scale -> project ----
        u8 = big.tile([128, 2, NQ], f8)
        for nch in range(2):
            ns = slice(nch * 512, (nch + 1) * 512)
            rp = pr.tile([128, 512], f32, name="rp", tag="r")
            up = [pacc.tile([128, 512], f32, name=f"up{h}", tag="u")
                  for h in range(2)]
            for s in range(16):
                if nch == 0 and s == 2:
                    emit_q(1, chans=(0,))
                if nch == 0 and s == 4:
                    emit_q(1, chans=(1,))
                if nch == 1 and s == 2 and nch_tail[0] is not None:
                    nch_tail[0]()          # nch0's normalize/project/store
                sp = psp.tile([128, 1024], f32, name="sp", tag="sp")
                for h in range(2):
                    ms = slice((2 * s + h) * 128, (2 * s + h + 1) * 128)
                    hs = slice(512 * h, 512 * (h + 1))
                    nc.tensor.matmul(sp[:, hs], lhsT=xb8[:, :, ms],
                                     rhs=q8[:, :, ns], start=True, stop=True,
                                     perf_mode=DR)
                e = epool.tile([128, 2, 512], f8, name="e", tag="e")
                nc.scalar.activation(out=e, in_=sp, func=AF.Exp,
                                     scale=1.0 / 16.0, bias=nbias)
                nc.tensor.matmul(rp, lhsT=ones8, rhs=e,
                                 start=(s == 0), stop=(s == 15), perf_mode=DR)
                for h in range(2):
                    nc.tensor.matmul(
                        up[h],
                        lhsT=xt8[:, 2 * s:2 * s + 2, 136 * h:136 * h + 128],
                        rhs=e, start=(s == 0), stop=(s == 15), perf_mode=DR)
            rb = work.tile([128, 512], f32, name="rb", tag="rb", bufs=2)
            nc.vector.reciprocal(out=rb, in_=rp)
            for h in range(2):
                nc.vector.tensor_mul(u8[:, h, ns], up[h], rb)

            # fused projection; out = pp/64 + (xq + bpe) in one stt
            for ot in range(2):
                if nch == 1:
                    pp = psp.tile([128, 512], f32, name="pp", tag="sp")
                else:
                    pp = pw.tile([128, 512], f32, name="pp", tag="pw")
                nc.tensor.matmul(pp, lhsT=pva8[:, :, ot * 128:(ot + 1) * 128],
                                 rhs=u8[:, :, ns], start=True, stop=True,
                                 perf_mode=DR)
                ot_t = work.tile([128, 512], bf16, name="ot_t", tag="ot_t")
                nc.vector.scalar_tensor_tensor(out=ot_t, in0=pp,
                                               scalar=1.0 / 64.0,
                                               in1=xqb[:, ot, ns], op0=MULT,
                                               op1=ADD)
                nc.sync.dma_start(out=out_d[:, NQ * ot + 512 * nch:
                                            NQ * ot + 512 * (nch + 1)],
                                  in_=ot_t)

    nc.compile()
    return nc


def _get_nc():
    key = "nc"
    if key not in _CACHE:
        _CACHE[key] = _build_nc()
    return _CACHE[key]


def _host_inputs(x, gamma, beta, Wq, bq, Wk, bk, Wv, bv, Wp, bp):
    import ml_dtypes
    f8 = ml_dtypes.float8_e4m3
    bf = ml_dtypes.bfloat16

    x = np.asarray(x, np.float32)
    xf = np.ascontiguousarray(x.reshape(2, C, N))
    gamma = np.asarray(gamma, np.float32)
    beta = np.asarray(beta, np.float32)
    Wq, Wk, Wv, Wp = [np.asarray(W, np.float32) for W in (Wq, Wk, Wv, Wp)]
    bq, bv, bp = [np.asarray(v, np.float32) for v in (bq, bv, bp)]

    # host-fused matrices: G = Wk^T Wq, Pv = Wp @ Wv
    G = Wk.T @ Wq                                             # [c, c']
    Pv = Wp @ Wv                                              # [o, c]
    # gt[p, i, c] = G[c, i*128+p]  (contraction over c' = i*128+p)
    gt = np.ascontiguousarray(
        G.T.reshape(2, 128, C).transpose(1, 0, 2).reshape(128, 2 * C)
    ).astype(bf)
    # pvt[p, i, o] = Pv[o, i*128+p]; the r-matmul's 1/64 stationary makes
    # u8 = 64*u/r (fp8 normal range), repaid by the epilogue's /64.
    pvt = np.ascontiguousarray(
        Pv.T.reshape(2, 128, C).transpose(1, 0, 2).reshape(128, 2 * C)
    ).astype(bf)
    # wall[p, i, (wv|wp), o] = W[o, i*128+p]
    wall = np.stack([Wv.T, Wp.T], axis=1)                     # [c, 2, o]
    wall = wall.reshape(2, 128, 2, C).transpose(1, 0, 2, 3)
    wall = np.ascontiguousarray(wall.reshape(128, 2 * 2 * C)).astype(bf)

    small = np.zeros((128, 26), np.float32)
    small[:, 0:2] = gamma.reshape(2, 128).T
    small[:, 2:4] = beta.reshape(2, 128).T
    small[:, 4:6] = bv.reshape(2, 128).T
    small[:, 6:8] = (Wk.T @ bq).reshape(2, 128).T
    small[:, 8:10] = bp.reshape(2, 128).T
    cids = np.arange(C)
    gm = np.zeros((C, GROUPS), np.float32)
    gm[cids, cids // GSIZE] = 1.0                             # [c, g]
    gmT = gm.reshape(2, 128, GROUPS).transpose(1, 0, 2)       # [p, i, g]
    small[:, 10:18] = gmT[:, 0, :] / (GSIZE * N)
    small[:, 18:26] = gmT[:, 1, :] / (GSIZE * N)
    gmask8 = np.ascontiguousarray(gm.T)                       # [g, c]
    imask = np.eye(128, dtype=np.float32).astype(bf)

    in_maps = []
    for core in range(8):
        b, j = divmod(core, 4)
        xrot = np.roll(xf[b], -j * NQ, axis=1)                # [C, N]
        x8 = xrot.astype(f8)
        xb8 = np.ascontiguousarray(
            x8.reshape(2, 128, N).transpose(1, 0, 2).reshape(128, 2 * N))
        dxq = (xrot[:, :NQ] - x8[:, :NQ].astype(np.float32))
        dxq = np.ascontiguousarray(
            dxq.reshape(2, 128, NQ).transpose(1, 0, 2).reshape(128, 2 * NQ)
        ).astype(bf)
        # xt8: position-major with ones cols: per n: [c0..c127, 1, c128.., 1]
        xt = x8.astype(np.float32).T                          # [n, c] quantized
        arr = np.zeros((N, 272), np.float32)
        arr[:, 0:128] = xt[:, 0:128]
        arr[:, 128] = 1.0
        arr[:, 136:264] = xt[:, 128:256]
        arr[:, 264] = 1.0
        xt8 = np.ascontiguousarray(
            arr.reshape(32, 128, 272).transpose(1, 0, 2).reshape(128, 32 * 272)
        ).astype(f8)
        in_maps.append({
            "xt8": xt8, "xb8": xb8, "gt": gt, "pvt": pvt, "wall": wall,
            "small": small, "gmask8": gmask8, "imask": imask,
            "dxq": dxq,
        })
    return in_maps


def kernel(x, gamma, beta, Wq, bq, Wk, bk, Wv, bv, Wp, bp):
    from concourse.bass_utils import run_bass_kernel_spmd
    global LAST_RESULTS

    orig_shape = np.asarray(x).shape
    in_maps = _host_inputs(x, gamma, beta, Wq, bq, Wk, bk, Wv, bv, Wp, bp)
    nc = _get_nc()

    trace = os.environ.get("BASSK_TRACE", "0") == "1"
    res = run_bass_kernel_spmd(nc, in_maps, core_ids=list(range(8)), trace=trace)
    LAST_RESULTS = res

    out = np.empty((2, C, N), np.float32)
    for core in range(8):
        b, j = divmod(core, 4)
        o = res.results[core]["out"].astype(np.float32)       # [128, 2*NQ]
        o = o.reshape(128, 2, NQ).transpose(1, 0, 2).reshape(C, NQ)
        out[b][:, j * NQ:(j + 1) * NQ] = o
    return out.reshape(orig_shape)
